# revision 1
# baseline (speedup 1.0000x reference)
"""ASAP-GNN classifier on 8 trn2 NeuronCores.

Per layer: GraphConv (slot-gather + slot-sum + GEMM) -> ASAPool (slot-gather +
slot-max / attention softmax over slots + weighted slot-sum + fused dot heads)
-> host top-k selection -> device kNN (K=4 dist matmul + max8/max_index).

Distribution: dst-node sharding across 8 cores; feature matrices are
replicated to every core's HBM between launches (host acts as interconnect).
Host does only index plumbing: edge sort into a fixed-slot layout, scalar
segment-sum for the LEConv fitness, top-k, final 1x1536 MLP.
"""

import math
import numpy as np

N0 = 20000
IN_CH = 64
HID = 512
OUT = 10
L = 3
RATIO = 0.5
NCORES = 8

DEV_CONV = True
DEV_POOL = True
DEV_KNN = True

_f32 = np.float32


# ----------------------------------------------------------------------------
# host helpers
# ----------------------------------------------------------------------------

def _pad_to(a, n, axis=0, fill=0):
    pad = n - a.shape[axis]
    if pad <= 0:
        return a
    widths = [(0, 0)] * a.ndim
    widths[axis] = (0, pad)
    return np.pad(a, widths, constant_values=fill)


def _slot_tables(src, dst, npad, sentinel):
    """Slot table [npad, D]: row i lists the src of i's in-edges (sentinel pad).
    Also returns valid mask and deg."""
    deg = np.bincount(dst, minlength=npad).astype(np.int64)
    D = max(1, int(deg.max()))
    order = np.argsort(dst, kind="stable")
    ss = src[order]
    ds = dst[order]
    # slot position of each edge within its dst segment
    starts = np.zeros(npad + 1, np.int64)
    np.cumsum(deg, out=starts[1:])
    slot = np.arange(len(ds)) - starts[ds]
    tbl = np.full((npad, D), sentinel, np.int64)
    tbl[ds, slot] = ss
    valid = np.zeros((npad, D), bool)
    valid[ds, slot] = True
    return tbl, valid, deg, D


def _slotmajor(tbl_core):
    """[rows, D] -> slot-major logical idx list per 128-block."""
    rows, D = tbl_core.shape
    out = np.empty(rows * D, np.int64)
    o = 0
    for b in range(rows // 128):
        blk = tbl_core[b * 128:(b + 1) * 128]  # [128, D]
        out[o:o + 128 * D] = blk.T.ravel()
        o += 128 * D
    return out


def _idx_to_i16_tile(idx_list):
    """dma_gather idx layout: element m -> partition m%16, col m//16,
    replicated across the 8 Q7 groups."""
    n = len(idx_list)
    S = (n + 15) // 16
    a = np.full((S, 16), -1, np.int16)
    a.reshape(-1)[:n] = idx_list.astype(np.int16)
    return np.ascontiguousarray(np.tile(a.T, (8, 1)))


def _degree_layout(src_a, dst_a, npad, sentinel):
    """Degree-sorted striped block layout shared by all cores.

    Returns (tbl, valid, deg, core_rows, Ds): core_rows[c] = original node ids
    of core c's rows (position-major); Ds[p] = slot count of every core's p-th
    block (max over the 8 striped blocks at that position)."""
    tbl, valid, deg, D = _slot_tables(src_a, dst_a, npad, sentinel)
    order = np.argsort(deg, kind="stable")
    nb = npad // 128
    BPC = nb // NCORES
    Dr = [max(1, int(deg[order[r * 128:(r + 1) * 128]].max())) for r in range(nb)]
    Ds = [max(Dr[NCORES * p:NCORES * (p + 1)]) for p in range(BPC)]
    core_rows = []
    for c in range(NCORES):
        core_rows.append(np.concatenate(
            [order[(NCORES * p + c) * 128:(NCORES * p + c + 1) * 128]
             for p in range(BPC)]))
    return tbl, valid, deg, core_rows, Ds


def _slot_idx_list(tbl, rows_idx, Ds):
    """Concatenated slot-major gather list for one core."""
    parts = []
    for p, D in enumerate(Ds):
        blk = tbl[rows_idx[p * 128:(p + 1) * 128], :D]  # [128, D]
        parts.append(blk.T.ravel())
    return np.concatenate(parts)


# ----------------------------------------------------------------------------
# numpy fallbacks
# ----------------------------------------------------------------------------

def _conv_np(x, src, dst, n, wr, br, wl):
    agg = np.zeros((n, wr.shape[0]), _f32)
    np.add.at(agg, dst, x[src])
    deg = np.bincount(dst, minlength=n).astype(_f32)
    mean = agg / np.maximum(deg, 1.0)[:, None]
    return np.maximum(mean @ wr + br + x @ wl, 0.0).astype(_f32)


def _pool_np(h, src, dst, n, lw, lb, aw, ab, l1w, l2w, l3w):
    sl = np.arange(n)
    s = np.concatenate([src, sl])
    d = np.concatenate([dst, sl])
    xj = h[s]
    xq = np.full((n, h.shape[1]), -np.inf, _f32)
    np.maximum.at(xq, d, xj)
    xqw = (xq @ lw + lb).astype(_f32)
    score = (xqw[d] @ aw[:HID] + xj @ aw[HID:] + ab).astype(_f32)
    score = np.where(score > 0, score, 0.2 * score).astype(_f32)
    smax = np.full(n, -np.inf, _f32)
    np.maximum.at(smax, d, score)
    ex = np.exp(score - smax[d])
    ssum = np.zeros(n, _f32)
    np.add.at(ssum, d, ex)
    att = (ex / ssum[d]).astype(_f32)
    xn = np.zeros_like(xq)
    np.add.at(xn, d, xj * att[:, None])
    abc = np.stack([xn @ l1w, xn @ l2w, xn @ l3w], 1).astype(_f32)
    return xn.astype(_f32), abc


def _knn_np(pos, k):
    n = pos.shape[0]
    sq = np.sum(pos * pos, axis=-1, dtype=_f32)
    dist = (sq[:, None] + sq[None, :] - 2.0 * (pos @ pos.T)).astype(_f32)
    np.fill_diagonal(dist, np.inf)
    idx = np.argsort(dist, axis=1, kind="stable")[:, :k]
    return idx.reshape(-1), np.repeat(np.arange(n), k)


# ----------------------------------------------------------------------------
# bass launches
# ----------------------------------------------------------------------------

_BASS = {}


def _get_bass():
    if not _BASS:
        import concourse.bass as bass
        import concourse.bacc as bacc
        import concourse.mybir as mybir
        from concourse.tile import TileContext
        from concourse.masks import make_identity
        from concourse import bass_utils
        _BASS.update(bass=bass, bacc=bacc, mybir=mybir, TileContext=TileContext,
                     bass_utils=bass_utils, make_identity=make_identity)
    return _BASS


_EXEC_NS = []


def _run_spmd(nc, in_maps, label=""):
    B = _get_bass()
    import time as _t
    t0 = _t.time()
    res = B["bass_utils"].run_bass_kernel_spmd(
        nc, in_maps, core_ids=list(range(NCORES)), trace=False)
    dt_ns = int((_t.time() - t0) * 1e9)
    _EXEC_NS.append((label, res.exec_time_ns or dt_ns))
    return res.results


def _ceil(a, b):
    return (a + b - 1) // b


def _build_conv_launch(F, Ds, BPC, nfeat):
    B = _get_bass()
    bass, mybir, TileContext = B["bass"], B["mybir"], B["TileContext"]
    dt = mybir.dt
    rows = BPC * 128
    S = 128 * sum(Ds) // 16
    KF = _ceil(F, 128)
    KW = _ceil(F + 1, 128)
    nc = B["bacc"].Bacc("TRN2", target_bir_lowering=False)
    feat = nc.dram_tensor("feat", [nfeat, F], dt.float32, kind="ExternalInput")
    featT = nc.dram_tensor("featT", [F, rows], dt.float32, kind="ExternalInput")
    gidx = nc.dram_tensor("gidx", [128, S], dt.int16, kind="ExternalInput")
    invdeg = nc.dram_tensor("invdeg", [rows, 1], dt.float32, kind="ExternalInput")
    wrb_c = nc.dram_tensor("wrb_c", [128, KW, HID], dt.float32, kind="ExternalInput")
    wl_c = nc.dram_tensor("wl_c", [128, KF, HID], dt.float32, kind="ExternalInput")
    h_out = nc.dram_tensor("h", [rows, HID], dt.float32, kind="ExternalOutput")

    with TileContext(nc) as tc:
        with (
            tc.tile_pool(name="const", bufs=1) as cpool,
            tc.tile_pool(name="gath", bufs=3) as gpool,
            tc.tile_pool(name="work", bufs=3) as wpool,
            tc.tile_pool(name="tps", bufs=2, space="PSUM") as tpool,
            tc.tile_pool(name="hps", bufs=2, space="PSUM") as hpool,
        ):
            ident = cpool.tile([128, 128], dt.float32)
            B["make_identity"](nc, ident[:])
            onesc = cpool.tile([128, 128], dt.float32)
            nc.vector.memset(onesc[:], 0.0)
            nc.vector.memset(onesc[0:1, :], 1.0)
            wrb_sb = cpool.tile([128, KW, HID], dt.float32)
            nc.sync.dma_start(wrb_sb[:], wrb_c[:, :, :])
            wl_sb = cpool.tile([128, KF, HID], dt.float32)
            nc.sync.dma_start(wl_sb[:], wl_c[:, :, :])
            idx_sb = cpool.tile([128, S], dt.int16)
            nc.sync.dma_start(idx_sb[:], gidx[:, :])

            single = (F + 1) <= 128  # ones row shares chunk 0
            idx_off = 0
            for b in range(BPC):
                D = Ds[b]
                r0, r1 = b * 128, (b + 1) * 128
                g = gpool.tile([128, D, F], dt.float32, tag="g")
                nc.gpsimd.dma_gather(
                    out_ap=g[:], in_ap=feat[:, :],
                    idxs_ap=idx_sb[:, idx_off // 16:(idx_off + 128 * D) // 16],
                    num_idxs=128 * D, num_idxs_reg=128 * D, elem_size=F,
                    single_packet=False)
                idx_off += 128 * D
                acc = wpool.tile([128, F], dt.float32, tag="acc")
                if D == 1:
                    nc.vector.tensor_copy(acc[:], g[:, 0, :])
                else:
                    nc.vector.tensor_add(acc[:], g[:, 0, :], g[:, 1, :])
                    for s_ in range(2, D):
                        nc.vector.tensor_add(acc[:], acc[:], g[:, s_, :])
                iv = wpool.tile([128, 1], dt.float32, tag="iv")
                nc.sync.dma_start(iv[:], invdeg[r0:r1, :])
                nc.vector.tensor_scalar_mul(acc[:], acc[:], iv[:])
                meanT = wpool.tile([128, KF, 128], dt.float32, tag="meanT")
                if single:
                    nc.vector.memset(meanT[:], 0.0)
                for fc in range(KF):
                    f0, f1 = fc * 128, min(F, (fc + 1) * 128)
                    tp = tpool.tile([128, 128], dt.float32, tag="tp")
                    nc.tensor.transpose(tp[:f1 - f0, :], acc[:, f0:f1], ident[:])
                    nc.vector.tensor_copy(meanT[0:f1 - f0, fc, :], tp[:f1 - f0, :])
                if single:
                    nc.vector.memset(meanT[F:F + 1, 0, :], 1.0)
                hps = hpool.tile([128, HID], dt.float32, tag="h")
                for fc in range(KF):
                    nc.tensor.matmul(hps[:], meanT[:, fc, :], wrb_sb[:, fc, :],
                                     start=(fc == 0), stop=False)
                if not single:
                    nc.tensor.matmul(hps[:], onesc[:], wrb_sb[:, KW - 1, :],
                                     start=False, stop=False)
                xT = wpool.tile([128, KF, 128], dt.float32, tag="xT")
                for fc in range(KF):
                    f0, f1 = fc * 128, min(F, (fc + 1) * 128)
                    nc.sync.dma_start(xT[0:f1 - f0, fc, :], featT[f0:f1, r0:r1])
                    nc.tensor.matmul(hps[:], xT[0:f1 - f0, fc, :],
                                     wl_sb[0:f1 - f0, fc, :],
                                     start=False, stop=(fc == KF - 1))
                hsb = wpool.tile([128, HID], dt.float32, tag="hsb")
                nc.scalar.activation(hsb[:], hps[:],
                                     mybir.ActivationFunctionType.Relu)
                nc.sync.dma_start(h_out[r0:r1, :], hsb[:])
    nc.compile()
    return nc


def _conv_dev(x, src, dst, n, wr, br, wl, aw2):
    BPC = _ceil(n, NCORES * 128)
    rows = BPC * 128
    npad = rows * NCORES
    F = x.shape[1]
    sentinel = n
    feat = np.ascontiguousarray(np.concatenate([x, np.zeros((1, F), _f32)], 0))
    tbl, valid, deg, core_rows, Ds = _degree_layout(src, dst, npad, sentinel)
    invdeg = (1.0 / np.maximum(deg, 1.0)).astype(_f32)
    xpadT = np.ascontiguousarray(_pad_to(x, npad).T)
    KF = _ceil(F, 128)
    KW = _ceil(F + 1, 128)
    wrb_pad = np.zeros((KW * 128, HID), _f32)
    wrb_pad[:F] = wr
    wrb_pad[F if KW == 1 else (KW - 1) * 128] = br
    wrb_c = np.ascontiguousarray(
        wrb_pad.reshape(KW, 128, HID).transpose(1, 0, 2))
    wl_pad = np.zeros((KF * 128, HID), _f32)
    wl_pad[:F] = wl
    wl_c = np.ascontiguousarray(wl_pad.reshape(KF, 128, HID).transpose(1, 0, 2))
    nc = _build_conv_launch(F, Ds, BPC, feat.shape[0])
    in_maps = []
    for c in range(NCORES):
        ri = core_rows[c]
        in_maps.append({
            "feat": feat,
            "featT": np.ascontiguousarray(xpadT[:, ri]),
            "gidx": _idx_to_i16_tile(_slot_idx_list(tbl, ri, Ds)),
            "invdeg": np.ascontiguousarray(invdeg[ri, None]),
            "wrb_c": wrb_c,
            "wl_c": wl_c,
        })
    outs = _run_spmd(nc, in_maps, "conv")
    h = np.empty((npad, HID), _f32)
    for c in range(NCORES):
        h[core_rows[c]] = outs[c]["h"]
    h = np.ascontiguousarray(h[:n])
    js = (h @ aw2).astype(_f32)
    return h, js


def _build_pool_launch(F, Ds, Dmax, BPC, nfeat, QB):
    B = _get_bass()
    bass, mybir, TileContext = B["bass"], B["mybir"], B["TileContext"]
    dt = mybir.dt
    rows = BPC * 128
    D = Dmax  # jslot input width
    S = 128 * sum(Ds) // 16
    nc = B["bacc"].Bacc("TRN2", target_bir_lowering=False)
    feat = nc.dram_tensor("feat", [nfeat, F], dt.float32, kind="ExternalInput")
    gidx = nc.dram_tensor("gidx", [128, S], dt.int16, kind="ExternalInput")
    jslot = nc.dram_tensor("jslot", [rows, D], dt.float32, kind="ExternalInput")
    qwc = nc.dram_tensor("qwc", [128, F // 128], dt.float32, kind="ExternalInput")
    xn_out = nc.dram_tensor("xn", [rows, F], dt.float32, kind="ExternalOutput")
    qs_out = nc.dram_tensor("qs", [rows, 1], dt.float32, kind="ExternalOutput")

    with TileContext(nc) as tc:
        with (
            tc.tile_pool(name="const", bufs=1) as cpool,
            tc.tile_pool(name="gath", bufs=3) as gpool,
            tc.tile_pool(name="work", bufs=3) as wpool,
            tc.tile_pool(name="tps", bufs=2, space="PSUM") as tpool,
            tc.tile_pool(name="qps", bufs=2, space="PSUM") as qpool,
        ):
            ident = cpool.tile([128, 128], dt.float32)
            B["make_identity"](nc, ident[:])
            qw_sb = cpool.tile([128, F // 128], dt.float32)
            nc.sync.dma_start(qw_sb[:], qwc[:, :])
            idx_sb = cpool.tile([128, S], dt.int16)
            nc.sync.dma_start(idx_sb[:], gidx[:, :])

            idx_off = 0
            for b in range(BPC):
                D = Ds[b]
                r0, r1 = b * 128, (b + 1) * 128
                g = gpool.tile([128, D, F], dt.float32, tag="g")
                nc.gpsimd.dma_gather(
                    out_ap=g[:], in_ap=feat[:, :],
                    idxs_ap=idx_sb[:, idx_off // 16:(idx_off + 128 * D) // 16],
                    num_idxs=128 * D, num_idxs_reg=128 * D, elem_size=F,
                    single_packet=False)
                idx_off += 128 * D
                xq = wpool.tile([128, F], dt.float32, tag="xq")
                if D == 1:
                    nc.vector.tensor_copy(xq[:], g[:, 0, :])
                else:
                    nc.vector.tensor_max(xq[:], g[:, 0, :], g[:, 1, :])
                    for s_ in range(2, D):
                        nc.vector.tensor_max(xq[:], xq[:], g[:, s_, :])
                qps = qpool.tile([128, 1], dt.float32, tag="qps")
                xqT = wpool.tile([128, 128], dt.float32, tag="xqT")
                KF = F // 128
                for fc in range(KF):
                    tp = tpool.tile([128, 128], dt.float32, tag="tp")
                    nc.tensor.transpose(tp[:], xq[:, fc * 128:(fc + 1) * 128],
                                        ident[:])
                    nc.vector.tensor_copy(xqT[:], tp[:])
                    nc.tensor.matmul(qps[:], xqT[:], qw_sb[:, fc:fc + 1],
                                     start=(fc == 0), stop=(fc == KF - 1))
                qsb = wpool.tile([128, 1], dt.float32, tag="qsb")
                nc.vector.tensor_copy(qsb[:], qps[:])
                nc.sync.dma_start(qs_out[r0:r1, :], qsb[:])
                js_t = wpool.tile([128, D], dt.float32, tag="js")
                nc.sync.dma_start(js_t[:], jslot[r0:r1, 0:D])
                qsb2 = wpool.tile([128, 1], dt.float32, tag="qsb2")
                nc.vector.tensor_scalar(qsb2[:], qsb[:], float(QB[0]), None,
                                        op0=mybir.AluOpType.add)
                sc = wpool.tile([128, D], dt.float32, tag="sc")
                nc.vector.tensor_scalar_add(sc[:], js_t[:], qsb2[:])
                sc2 = wpool.tile([128, D], dt.float32, tag="sc2")
                nc.vector.tensor_scalar(sc2[:], sc[:], 0.2, None,
                                        op0=mybir.AluOpType.mult)
                nc.vector.tensor_max(sc[:], sc[:], sc2[:])
                m = wpool.tile([128, 1], dt.float32, tag="m")
                nc.vector.tensor_reduce(m[:], sc[:], axis=mybir.AxisListType.X,
                                        op=mybir.AluOpType.max)
                nc.vector.tensor_scalar(sc[:], sc[:], m[:], None,
                                        op0=mybir.AluOpType.subtract)
                nc.scalar.activation(sc[:], sc[:],
                                     mybir.ActivationFunctionType.Exp)
                ssum = wpool.tile([128, 1], dt.float32, tag="ssum")
                nc.vector.tensor_reduce(ssum[:], sc[:], axis=mybir.AxisListType.X,
                                        op=mybir.AluOpType.add)
                rec = wpool.tile([128, 1], dt.float32, tag="rec")
                nc.vector.reciprocal(rec[:], ssum[:])
                nc.vector.tensor_scalar_mul(sc[:], sc[:], rec[:])
                xn = wpool.tile([128, F], dt.float32, tag="xn")
                nc.vector.tensor_scalar_mul(xn[:], g[:, 0, :], sc[:, 0:1])
                for s_ in range(1, D):
                    nc.vector.scalar_tensor_tensor(
                        out=xn[:], in0=g[:, s_, :], scalar=sc[:, s_:s_ + 1],
                        in1=xn[:], op0=mybir.AluOpType.mult,
                        op1=mybir.AluOpType.add)
                nc.sync.dma_start(xn_out[r0:r1, :], xn[:])
    nc.compile()
    return nc


def _pool_dev(h, src, dst, n, lw, lb, aw, ab, js):
    sl = np.arange(n)
    s_all = np.concatenate([src, sl])
    d_all = np.concatenate([dst, sl])
    BPC = _ceil(n, NCORES * 128)
    rows = BPC * 128
    npad = rows * NCORES
    sentinel = n
    feat = np.ascontiguousarray(np.concatenate([h, np.zeros((1, HID), _f32)], 0))
    tbl, valid, deg, core_rows, Ds = _degree_layout(s_all, d_all, npad, sentinel)
    Dmax = max(Ds)
    wq = (lw @ aw[:HID]).astype(_f32)
    qwc = np.ascontiguousarray(wq.reshape(HID // 128, 128).T, dtype=_f32)
    qb = float(lb @ aw[:HID] + ab)
    js_pad = _pad_to(js.astype(_f32), npad + 1)
    jslot = np.where(valid, js_pad[tbl], -1e30).astype(_f32)
    nc = _build_pool_launch(HID, Ds, Dmax, BPC, feat.shape[0], (qb,))
    in_maps = []
    for c in range(NCORES):
        ri = core_rows[c]
        in_maps.append({
            "feat": feat,
            "gidx": _idx_to_i16_tile(_slot_idx_list(tbl, ri, Ds)),
            "jslot": np.ascontiguousarray(jslot[ri][:, :Dmax]),
            "qwc": qwc,
        })
    outs = _run_spmd(nc, in_maps, "pool")
    xn_full = np.empty((npad, HID), _f32)
    for c in range(NCORES):
        xn_full[core_rows[c]] = outs[c]["xn"]
    xn = np.ascontiguousarray(xn_full[:n])
    l1w, l2w, l3w = _pool_dev._w3
    abc = np.stack([xn @ l1w, xn @ l2w, xn @ l3w], 1).astype(_f32)
    return xn, abc


def _build_knn_launch(BQ, ncand, two_rounds):
    B = _get_bass()
    bass, mybir, TileContext = B["bass"], B["mybir"], B["TileContext"]
    dt = mybir.dt
    NCH = ncand // 512
    nc = B["bacc"].Bacc("TRN2", target_bir_lowering=False)
    qT = nc.dram_tensor("qT", [4, BQ * 128], dt.float32, kind="ExternalInput")
    cand = nc.dram_tensor("cand", [4, ncand], dt.float32, kind="ExternalInput")
    iout = nc.dram_tensor("idx8", [BQ * 128, 8], dt.uint32, kind="ExternalOutput")
    iout2 = (nc.dram_tensor("idx8b", [BQ * 128, 8], dt.uint32,
                            kind="ExternalOutput") if two_rounds else None)
    with TileContext(nc) as tc:
        with (
            tc.tile_pool(name="const", bufs=1) as cpool,
            tc.tile_pool(name="rowb", bufs=2) as rpool,
            tc.tile_pool(name="ps", bufs=4, space="PSUM") as pspool,
            tc.tile_pool(name="sm", bufs=3) as spool,
        ):
            cand_sb = cpool.tile([4, ncand], dt.float32)
            nc.sync.dma_start(cand_sb[:], cand[:, :])
            for b in range(BQ):
                qsb = spool.tile([4, 128], dt.float32, tag="q")
                nc.sync.dma_start(qsb[:], qT[:, b * 128:(b + 1) * 128])
                row = rpool.tile([128, ncand], dt.float32, tag="row")
                for ch in range(NCH):
                    dps = pspool.tile([128, 512], dt.float32, tag="d")
                    nc.tensor.matmul(dps[:], qsb[:],
                                     cand_sb[:, ch * 512:(ch + 1) * 512],
                                     start=True, stop=True)
                    nc.scalar.activation(row[:, ch * 512:(ch + 1) * 512], dps[:],
                                         mybir.ActivationFunctionType.Copy)
                v8 = spool.tile([128, 8], dt.float32, tag="v8")
                nc.vector.max(out=v8[:], in_=row[:])
                i8 = spool.tile([128, 8], dt.uint32, tag="i8")
                nc.vector.max_index(i8[:], v8[:], row[:])
                nc.sync.dma_start(iout[b * 128:(b + 1) * 128, :], i8[:])
                if two_rounds:
                    nc.vector.match_replace(out=row[:], in_to_replace=v8[:],
                                            in_values=row[:], imm_value=-2e30)
                    v8b = spool.tile([128, 8], dt.float32, tag="v8b")
                    nc.vector.max(out=v8b[:], in_=row[:])
                    i8b = spool.tile([128, 8], dt.uint32, tag="i8b")
                    nc.vector.max_index(i8b[:], v8b[:], row[:])
                    nc.sync.dma_start(iout2[b * 128:(b + 1) * 128, :], i8b[:])
    nc.compile()
    return nc


def _knn_dev(pos, k):
    n = pos.shape[0]
    BQ = _ceil(n, NCORES * 128)
    nq_pc = BQ * 128
    ncand = _ceil(n, 512) * 512
    pos = pos.astype(_f32)
    sq = np.sum(pos * pos, axis=-1, dtype=_f32)
    cand = np.zeros((4, ncand), _f32)
    cand[0, :n] = pos[:, 0]
    cand[1, :n] = pos[:, 1]
    cand[2, :n] = sq
    cand[3, :] = 1.0
    cand[2, n:] = 1e30
    two_rounds = k >= 8
    nc = _build_knn_launch(BQ, ncand, two_rounds)
    in_maps = []
    for c in range(NCORES):
        qTv = np.zeros((4, nq_pc), _f32)
        lo = c * nq_pc
        hi = min(n, lo + nq_pc)
        if hi > lo:
            m = hi - lo
            qTv[0, :m] = 2.0 * pos[lo:hi, 0]
            qTv[1, :m] = 2.0 * pos[lo:hi, 1]
            qTv[2, :m] = -1.0
            qTv[3, :m] = -sq[lo:hi]
        in_maps.append({"qT": qTv, "cand": cand})
    outs = _run_spmd(nc, in_maps, "knn")
    cand8 = np.concatenate([o["idx8"] for o in outs], 0)[:n].astype(np.int64)
    if two_rounds:
        cand8b = np.concatenate([o["idx8b"] for o in outs], 0)[:n].astype(np.int64)
        cand8 = np.concatenate([cand8, cand8b], 1)
    # host: drop self, validate, per-row fallback
    idx = np.empty((n, k), np.int64)
    selfid = np.arange(n)
    fallback = 0
    for i in range(n):
        row = cand8[i]
        keep = row[row != i][:k + 2]
        uniq = len(set(keep.tolist())) == len(keep)
        if len(keep) >= k and uniq:
            idx[i] = keep[:k]
        else:
            d = sq + sq[i] - 2.0 * (pos @ pos[i])
            d[i] = np.inf
            idx[i] = np.argsort(d, kind="stable")[:k]
            fallback += 1
    if fallback:
        print(f"knn host fallback rows: {fallback}")
    return idx.reshape(-1), np.repeat(np.arange(n), k)


# ----------------------------------------------------------------------------
# main kernel
# ----------------------------------------------------------------------------

def kernel(x, pos, edge_index, conv0_wr, conv0_br, conv0_wl, conv_wr, conv_br,
           conv_wl, pool_lin_w, pool_lin_b, pool_att_w, pool_att_b, le1_w,
           le1_b, le2_w, le3_w, le3_b, lin1_w, lin1_b, lin2_w, lin2_b):
    x = np.asarray(x, _f32)
    pos = np.asarray(pos, _f32)
    ei = np.asarray(edge_index).astype(np.int64)
    src, dst = ei[0], ei[1]
    n = N0
    _EXEC_NS.clear()
    xs = []
    for i in range(L):
        wr = np.asarray(conv0_wr if i == 0 else conv_wr[i - 1], _f32)
        br = np.asarray(conv0_br if i == 0 else conv_br[i - 1], _f32)
        wl = np.asarray(conv0_wl if i == 0 else conv_wl[i - 1], _f32)
        aw = np.asarray(pool_att_w[i], _f32)
        ab = float(pool_att_b[i])
        lw = np.asarray(pool_lin_w[i], _f32)
        lb = np.asarray(pool_lin_b[i], _f32)
        l1w, l1b = np.asarray(le1_w[i], _f32), float(le1_b[i])
        l2w = np.asarray(le2_w[i], _f32)
        l3w, l3b = np.asarray(le3_w[i], _f32), float(le3_b[i])

        if DEV_CONV:
            h, js = _conv_dev(x, src, dst, n, wr, br, wl, aw[HID:])
        else:
            h = _conv_np(x, src, dst, n, wr, br, wl)
            js = (h @ aw[HID:]).astype(_f32)

        if DEV_POOL:
            _pool_dev._w3 = (l1w, l2w, l3w)
            xn, abc = _pool_dev(h, src, dst, n, lw, lb, aw, ab, js)
        else:
            xn, abc = _pool_np(h, src, dst, n, lw, lb, aw, ab, l1w, l2w, l3w)

        sl = np.arange(n)
        s_all = np.concatenate([src, sl])
        d_all = np.concatenate([dst, sl])
        a = abc[:, 0] + l1b
        b_ = abc[:, 1]
        agg = np.zeros(n, _f32)
        np.add.at(agg, d_all, (a[s_all] - b_[d_all]).astype(_f32))
        z = (agg + abc[:, 2] + l3b).astype(_f32)

        k_keep = int(math.ceil(RATIO * n))
        fit64 = 1.0 / (1.0 + np.exp(-z.astype(np.float64)))
        perm = np.argpartition(-fit64, k_keep - 1)[:k_keep]
        perm.sort()
        fv = fit64[perm].astype(_f32)
        x = (xn[perm] * fv[:, None]).astype(_f32)
        xs.append(x.max(axis=0))
        pos = pos[perm]
        n = k_keep
        if i < L - 1:
            kk = 6 + 2 * i
            if DEV_KNN:
                src, dst = _knn_dev(pos, kk)
            else:
                src, dst = _knn_np(pos, kk)

    hcat = np.concatenate(xs)[None, :]
    h1 = np.maximum(hcat @ np.asarray(lin1_w, _f32) + np.asarray(lin1_b, _f32), 0)
    out = h1 @ np.asarray(lin2_w, _f32) + np.asarray(lin2_b, _f32)
    return out.astype(_f32)


def total_exec_ns():
    return sum(v for _, v in _EXEC_NS)


def exec_breakdown():
    return list(_EXEC_NS)



# revision 6
# speedup vs baseline: 1.5386x; 1.5386x over previous
"""ASAP-GNN classifier on trn2 via Bass/Tile.

Architecture (v2): single NeuronCore, device-resident features between
launches. Three compiled programs (NEFFs), built/compiled at import time in
background threads:

  L0  : layer-0 GraphConv + ASAPool attention + LEConv fitness over the
        irregular input graph (slot-table gathers, For_i loops over 157
        row-blocks of 128 nodes).
  L12 : same pipeline for layers 1 and 2 over the fixed-degree kNN graphs
        (shared program; layer-2's 5000 nodes padded to layer-1's shape).
  K   : top-half "select" (gather xn[perm]*fv -> next x + transposed copy +
        running global max) fused with the dense kNN distance scan
        (max8/max_index, two rounds -> 16 neighbor candidates).

Host does only: slot-table construction, top-k via argpartition on the
fitness logits, kNN candidate validation, and the final 1x1536 MLP. Per
layer one launch round-trip for fitness -> perm and one for select+kNN:
6 launches total, ~KBs of traffic each after the initial ~17MB upload.
"""

import math
import threading
import time
import numpy as np

N0 = 20000
IN_CH = 64
HID = 512
OUT = 10
L = 3
RATIO = 0.5

_f32 = np.float32

# ---- geometry constants (hardcoded; program shapes) ----
NB0 = 157                   # layer-0 row blocks
R0 = NB0 * 128              # 20096
X0_ROWS = R0 + 128          # feat_x0 rows (sentinel row = R0, zeros)
D0C_DEFAULT = 17            # layer-0 max in-degree (rebuilt if actual differs)

NB1 = 79                    # layer-1/2 row blocks
R1 = NB1 * 128              # 10112
D1C = 8                     # conv slots for kNN layers (k<=8)
D1P = 9                     # pool slots (self + 8)

XN_ROWS = 20352             # unified xn/x buffer rows (>= R0 + sentinel)
XT_COLS = 10240             # x1T columns (>= R1)
NCH = XT_COLS // 512        # kNN candidate chunks (20)


# ----------------------------------------------------------------------------
# bass plumbing
# ----------------------------------------------------------------------------

_BASS = {}


def _get_bass():
    if not _BASS:
        import concourse.bass as bass
        import concourse.bacc as bacc
        import concourse.mybir as mybir
        from concourse.tile import TileContext
        from concourse.masks import make_identity
        from concourse.bass import ds
        from concourse import bass2jax
        import jax
        import jax.numpy as jnp
        bass2jax.install_neuronx_cc_hook()
        _BASS.update(bass=bass, bacc=bacc, mybir=mybir, TileContext=TileContext,
                     make_identity=make_identity, ds=ds, bass2jax=bass2jax,
                     jax=jax, jnp=jnp)
    return _BASS


class _Launcher:
    """Compiled 1-core bass program; inputs/outputs stay jax device arrays."""

    def __init__(self, nc):
        B = _get_bass()
        jax, jnp, mybir = B["jax"], B["jnp"], B["mybir"]
        bass2jax = B["bass2jax"]
        partition_name = (nc.partition_id_tensor.name
                          if nc.partition_id_tensor else None)
        in_names, in_avals, out_names, out_avals = [], [], [], []
        for alloc in nc.m.functions[0].allocations:
            if not isinstance(alloc, mybir.MemoryLocationSet):
                continue
            name = alloc.memorylocations[0].name
            if alloc.kind == "ExternalInput":
                if name != partition_name:
                    in_names.append(name)
                    in_avals.append(jax.ShapeDtypeStruct(
                        tuple(alloc.tensor_shape), mybir.dt.np(alloc.dtype)))
            elif alloc.kind == "ExternalOutput":
                out_names.append(name)
                out_avals.append(jax.core.ShapedArray(
                    tuple(alloc.tensor_shape), mybir.dt.np(alloc.dtype)))
        self.in_names = in_names
        self.in_avals = in_avals
        self.out_names = out_names
        self.out_avals = out_avals
        n_params = len(in_names)
        all_names = in_names + out_names + (
            [partition_name] if partition_name else [])
        donate = tuple(range(n_params, n_params + len(out_names)))

        def _body(*args):
            operands = list(args)
            if partition_name is not None:
                operands.append(bass2jax.partition_id_tensor())
            outs = bass2jax._bass_exec_p.bind(
                *operands, out_avals=tuple(out_avals),
                in_names=tuple(all_names), out_names=tuple(out_names),
                lowering_input_output_aliases=(),
                sim_require_finite=True, sim_require_nnan=True, nc=nc)
            return tuple(outs)

        self._jit = jax.jit(_body, donate_argnums=donate, keep_unused=True)
        self._compiled = None

    def warm(self):
        """AOT-compile the executable (no execution)."""
        B = _get_bass()
        jax, jnp = B["jax"], B["jnp"]
        out_structs = [jax.ShapeDtypeStruct(av.shape, av.dtype)
                       for av in self.out_avals]
        self._compiled = self._jit.lower(*self.in_avals,
                                         *out_structs).compile()
        # warm the jnp.zeros broadcast kernels used for donated outputs
        for av in self.out_avals:
            jnp.zeros(av.shape, av.dtype).block_until_ready()
        return self

    def __call__(self, in_map):
        B = _get_bass()
        jnp = B["jnp"]
        args = [in_map[nm] for nm in self.in_names]
        zeros = [jnp.zeros(av.shape, av.dtype) for av in self.out_avals]
        fn = self._compiled if self._compiled is not None else self._jit
        outs = fn(*args, *zeros)
        return dict(zip(self.out_names, outs))


# ----------------------------------------------------------------------------
# host helpers
# ----------------------------------------------------------------------------

def _idx_to_i16_tile(idx_list):
    """dma_gather idx layout: element m -> partition m%16, col m//16,
    replicated across the 8 Q7 groups."""
    n = len(idx_list)
    S = (n + 15) // 16
    a = np.full((S, 16), -1, np.int16)
    a.reshape(-1)[:n] = idx_list.astype(np.int16)
    return np.ascontiguousarray(np.tile(a.T, (8, 1)))


def _slot_table(src, dst, nrows, D, sentinel):
    """[nrows, D] slot table: row i lists srcs of i's in-edges, sentinel pad."""
    deg = np.bincount(dst, minlength=nrows).astype(np.int64)
    order = np.argsort(dst, kind="stable")
    ss = src[order]
    dsrt = dst[order]
    starts = np.zeros(nrows + 1, np.int64)
    np.cumsum(deg, out=starts[1:])
    slot = np.arange(len(dsrt)) - starts[dsrt]
    tbl = np.full((nrows, D), sentinel, np.int64)
    tbl[dsrt, slot] = ss
    return tbl, deg


def _slotmajor_list(tbl):
    """[rows, D] -> block-slot-major gather list (per 128-block, slot-major)."""
    rows, D = tbl.shape
    nb = rows // 128
    return np.ascontiguousarray(
        tbl.reshape(nb, 128, D).transpose(0, 2, 1)).reshape(-1)


def _rep128(v):
    return np.full((128, 1), v, _f32)


# ----------------------------------------------------------------------------
# program builders
# ----------------------------------------------------------------------------

def _tree_sum(nc, g, n, view):
    """In-place binary-tree reduce over slot axis: view(g, lo, cnt) -> AP.
    Result lands in slot 0. Returns nothing."""
    w = n
    while w > 1:
        h = w // 2
        nc.vector.tensor_add(view(0, h), view(0, h), view(h, h))
        if w % 2:
            nc.vector.tensor_add(view(0, 1), view(0, 1), view(w - 1, 1))
        w = h


def _tree_max(nc, out_t, g, n, gview, oview):
    """Max over n slots of g into out_t (slot tile of n//2 width)."""
    h = n // 2
    nc.vector.tensor_max(oview(0, h), gview(0, h), gview(h, h))
    if n % 2:
        nc.vector.tensor_max(oview(0, 1), oview(0, 1), gview(n - 1, 1))
    w = h
    while w > 1:
        h2 = w // 2
        nc.vector.tensor_max(oview(0, h2), oview(0, h2), oview(h2, h2))
        if w % 2:
            nc.vector.tensor_max(oview(0, 1), oview(0, 1), oview(w - 1, 1))
        w = h2


def _build_L0(D0C):
    """Layer-0: conv + pool + fitness over irregular graph."""
    B = _get_bass()
    bacc, mybir, TileContext = B["bacc"], B["mybir"], B["TileContext"]
    ds, make_identity = B["ds"], B["make_identity"]
    dt = mybir.dt
    D0P = D0C + 1
    SC = NB0 * ((128 * D0C) // 16)       # conv idx tile cols
    SP = NB0 * ((128 * D0P) // 16)       # pool idx tile cols
    F = IN_CH
    HROWS = R0 + 128                     # feat_h rows, sentinel = R0

    nc = bacc.Bacc("TRN2", target_bir_lowering=False)
    x_d = nc.dram_tensor("x", [X0_ROWS, F], dt.float32, kind="ExternalInput")
    cidx_d = nc.dram_tensor("cidx", [128, SC], dt.int16, kind="ExternalInput")
    pidx_d = nc.dram_tensor("pidx", [128, SP], dt.int16, kind="ExternalInput")
    invdeg_d = nc.dram_tensor("invdeg", [R0, 1], dt.float32, kind="ExternalInput")
    cnt_d = nc.dram_tensor("cnt", [R0, 1], dt.float32, kind="ExternalInput")
    wxm_d = nc.dram_tensor("wxm", [128, HID], dt.float32, kind="ExternalInput")
    br_d = nc.dram_tensor("br", [128, HID], dt.float32, kind="ExternalInput")
    wq_d = nc.dram_tensor("wq", [128, HID], dt.float32, kind="ExternalInput")
    aw2_d = nc.dram_tensor("aw2", [128, HID], dt.float32, kind="ExternalInput")
    l1w_d = nc.dram_tensor("l1w", [128, HID], dt.float32, kind="ExternalInput")
    l2w_d = nc.dram_tensor("l2w", [128, HID], dt.float32, kind="ExternalInput")
    l3w_d = nc.dram_tensor("l3w", [128, HID], dt.float32, kind="ExternalInput")
    qb_d = nc.dram_tensor("qb", [128, 1], dt.float32, kind="ExternalInput")
    l1b_d = nc.dram_tensor("l1b", [128, 1], dt.float32, kind="ExternalInput")
    l3b_d = nc.dram_tensor("l3b", [128, 1], dt.float32, kind="ExternalInput")

    h_d = nc.dram_tensor("fh", [HROWS, 576], dt.float32, kind="Internal")
    a_d = nc.dram_tensor("fa", [HROWS, 64], dt.float32, kind="Internal")
    zb_d = nc.dram_tensor("zb", [R0, 1], dt.float32, kind="Internal")
    xn_d = nc.dram_tensor("xn", [XN_ROWS, HID], dt.float32,
                          kind="ExternalOutput")
    z_d = nc.dram_tensor("z", [R0, 1], dt.float32, kind="ExternalOutput")

    with TileContext(nc) as tc:
        with (
            tc.tile_pool(name="const", bufs=1) as cpool,
            tc.tile_pool(name="ps", bufs=2, space="PSUM") as pspool,
        ):
            ident = cpool.tile([128, 128], dt.float32)
            make_identity(nc, ident[:])
            wxm_sb = cpool.tile([128, HID], dt.float32)
            nc.sync.dma_start(wxm_sb[:], wxm_d[:, :])
            br_sb = cpool.tile([128, HID], dt.float32)
            nc.sync.dma_start(br_sb[:], br_d[:, :])
            wq_sb = cpool.tile([128, HID], dt.float32)
            nc.sync.dma_start(wq_sb[:], wq_d[:, :])
            aw2_sb = cpool.tile([128, HID], dt.float32)
            nc.sync.dma_start(aw2_sb[:], aw2_d[:, :])
            l1w_sb = cpool.tile([128, HID], dt.float32)
            nc.sync.dma_start(l1w_sb[:], l1w_d[:, :])
            l2w_sb = cpool.tile([128, HID], dt.float32)
            nc.sync.dma_start(l2w_sb[:], l2w_d[:, :])
            l3w_sb = cpool.tile([128, HID], dt.float32)
            nc.sync.dma_start(l3w_sb[:], l3w_d[:, :])
            qb_sb = cpool.tile([128, 1], dt.float32)
            nc.sync.dma_start(qb_sb[:], qb_d[:, :])
            l1b_sb = cpool.tile([128, 1], dt.float32)
            nc.sync.dma_start(l1b_sb[:], l1b_d[:, :])
            l3b_sb = cpool.tile([128, 1], dt.float32)
            nc.sync.dma_start(l3b_sb[:], l3b_d[:, :])
            pidx_sb = cpool.tile([128, SP], dt.int16)
            nc.sync.dma_start(pidx_sb[:], pidx_d[:, :])
            # sentinel rows: feat_h[R0] = zeros except js col = -1e30;
            # feat_a[R0] = 0
            srow = cpool.tile([1, 576], dt.float32)
            nc.vector.memset(srow[:], 0.0)
            nc.vector.memset(srow[:, 512:513], -1e30)
            nc.sync.dma_start(h_d[R0:R0 + 1, :], srow[:])
            nc.sync.dma_start(a_d[R0:R0 + 1, :], srow[:, 0:64])

            # ---- phase A: conv ----
            SCB = (128 * D0C) // 16
            with tc.tile_pool(name="conv", bufs=2) as wp:
                def conv_body(i):
                    ci = wp.tile([128, SCB], dt.int16, tag="ci")
                    nc.sync.dma_start(ci[:], cidx_d[:, ds(i * SCB, SCB)])
                    g = wp.tile([128, D0C, F], dt.float32, tag="g")
                    nc.gpsimd.dma_gather(
                        out_ap=g[:], in_ap=x_d[:, :], idxs_ap=ci[:],
                        num_idxs=128 * D0C, num_idxs_reg=128 * D0C,
                        elem_size=F, single_packet=False)
                    _tree_sum(nc, g, D0C,
                              lambda lo, cnt: g[:, lo:lo + cnt, :])
                    iv = wp.tile([128, 1], dt.float32, tag="iv")
                    nc.sync.dma_start(iv[:], invdeg_d[ds(i * 128, 128), :])
                    xm = wp.tile([128, 128], dt.float32, tag="xm")
                    nc.sync.dma_start(xm[:, 0:F], x_d[ds(i * 128, 128), :])
                    nc.vector.tensor_scalar_mul(xm[:, F:2 * F], g[:, 0, :],
                                                iv[:])
                    tp = pspool.tile([128, 128], dt.float32, tag="tp")
                    nc.tensor.transpose(tp[:], xm[:], ident[:])
                    lhsT = wp.tile([128, 128], dt.float32, tag="lhsT")
                    nc.vector.tensor_copy(lhsT[:], tp[:])
                    hps = pspool.tile([128, HID], dt.float32, tag="hps")
                    nc.tensor.matmul(hps[:], lhsT[:], wxm_sb[:],
                                     start=True, stop=True)
                    hsb = wp.tile([128, 576], dt.float32, tag="hsb")
                    nc.vector.tensor_add(
                        hsb[:, 0:HID], hps[:],
                        br_sb[:])
                    nc.vector.tensor_scalar_max(hsb[:, 0:HID], hsb[:, 0:HID],
                                                0.0)
                    tmp = wp.tile([128, HID], dt.float32, tag="tmp")
                    nc.vector.tensor_mul(tmp[:], hsb[:, 0:HID],
                                         aw2_sb[:])
                    nc.vector.tensor_reduce(hsb[:, 512:513], tmp[:],
                                            axis=mybir.AxisListType.X,
                                            op=mybir.AluOpType.add)
                    nc.sync.dma_start(h_d[ds(i * 128, 128), 0:513],
                                      hsb[:, 0:513])
                tc.For_i_unrolled(0, NB0, 1, conv_body, max_unroll=2)

            # ---- phase B: pool ----
            SPB = (128 * D0P) // 16
            with tc.tile_pool(name="pool", bufs=2) as wp:
                def pool_body(i):
                    g = wp.tile([128, D0P, 576], dt.float32, tag="g")
                    nc.gpsimd.dma_gather(
                        out_ap=g[:], in_ap=h_d[:, :],
                        idxs_ap=pidx_sb[:, ds(i * SPB, SPB)],
                        num_idxs=128 * D0P, num_idxs_reg=128 * D0P,
                        elem_size=576, single_packet=False)
                    xq = wp.tile([128, D0P // 2, HID], dt.float32, tag="xq")
                    _tree_max(nc, xq, g, D0P,
                              lambda lo, cnt: g[:, lo:lo + cnt, 0:HID],
                              lambda lo, cnt: xq[:, lo:lo + cnt, :])
                    tmp = wp.tile([128, HID], dt.float32, tag="tmp")
                    nc.vector.tensor_mul(tmp[:], xq[:, 0, :],
                                         wq_sb[:])
                    qs = wp.tile([128, 1], dt.float32, tag="qs")
                    nc.vector.tensor_reduce(qs[:], tmp[:],
                                            axis=mybir.AxisListType.X,
                                            op=mybir.AluOpType.add)
                    nc.vector.tensor_add(qs[:], qs[:], qb_sb[:])
                    # score = leaky_relu(qs + js)
                    sc = wp.tile([128, D0P], dt.float32, tag="sc")
                    jsv = g[:, :, 512:513].squeeze(2)
                    nc.vector.tensor_scalar_add(sc[:], jsv, qs[:])
                    sc2 = wp.tile([128, D0P], dt.float32, tag="sc2")
                    nc.vector.tensor_scalar_mul(sc2[:], sc[:], 0.2)
                    nc.vector.tensor_max(sc[:], sc[:], sc2[:])
                    m = wp.tile([128, 1], dt.float32, tag="m")
                    nc.vector.tensor_reduce(m[:], sc[:],
                                            axis=mybir.AxisListType.X,
                                            op=mybir.AluOpType.max)
                    nc.vector.tensor_scalar(sc[:], sc[:], m[:], None,
                                            op0=mybir.AluOpType.subtract)
                    nc.scalar.activation(sc[:], sc[:],
                                         mybir.ActivationFunctionType.Exp)
                    ssum = wp.tile([128, 1], dt.float32, tag="ssum")
                    nc.vector.tensor_reduce(ssum[:], sc[:],
                                            axis=mybir.AxisListType.X,
                                            op=mybir.AluOpType.add)
                    rec = wp.tile([128, 1], dt.float32, tag="rec")
                    nc.vector.reciprocal(rec[:], ssum[:])
                    nc.vector.tensor_scalar_mul(sc[:], sc[:], rec[:])
                    # xn = sum_s att_s * h_s  (scale slots in place, tree add)
                    gh = g[:, :, 0:HID]
                    nc.vector.tensor_mul(
                        gh, gh, sc[:].unsqueeze(2).to_broadcast(
                            [128, D0P, HID]))
                    _tree_sum(nc, g, D0P,
                              lambda lo, cnt: g[:, lo:lo + cnt, 0:HID])
                    xn = g[:, 0, 0:HID]
                    nc.sync.dma_start(xn_d[ds(i * 128, 128), :], xn)
                    # fitness scalars
                    nc.vector.tensor_mul(tmp[:], xn,
                                         l1w_sb[:])
                    av = wp.tile([128, 1], dt.float32, tag="av")
                    nc.vector.tensor_reduce(av[:], tmp[:],
                                            axis=mybir.AxisListType.X,
                                            op=mybir.AluOpType.add)
                    nc.sync.dma_start(a_d[ds(i * 128, 128), 0:1], av[:])
                    nc.vector.tensor_mul(tmp[:], xn,
                                         l2w_sb[:])
                    bv = wp.tile([128, 1], dt.float32, tag="bv")
                    nc.vector.tensor_reduce(bv[:], tmp[:],
                                            axis=mybir.AxisListType.X,
                                            op=mybir.AluOpType.add)
                    nc.vector.tensor_mul(tmp[:], xn,
                                         l3w_sb[:])
                    cv = wp.tile([128, 1], dt.float32, tag="cv")
                    nc.vector.tensor_reduce(cv[:], tmp[:],
                                            axis=mybir.AxisListType.X,
                                            op=mybir.AluOpType.add)
                    ct = wp.tile([128, 1], dt.float32, tag="ct")
                    nc.sync.dma_start(ct[:], cnt_d[ds(i * 128, 128), :])
                    # zb = c + l3b - cnt*b + cnt*l1b
                    zb = wp.tile([128, 1], dt.float32, tag="zb")
                    nc.vector.tensor_mul(zb[:], ct[:], bv[:])
                    nc.vector.tensor_sub(zb[:], cv[:], zb[:])
                    nc.vector.tensor_add(zb[:], zb[:], l3b_sb[:])
                    lb1 = wp.tile([128, 1], dt.float32, tag="lb1")
                    nc.vector.tensor_mul(lb1[:], ct[:], l1b_sb[:])
                    nc.vector.tensor_add(zb[:], zb[:], lb1[:])
                    nc.sync.dma_start(zb_d[ds(i * 128, 128), :], zb[:])
                tc.For_i_unrolled(0, NB0, 1, pool_body, max_unroll=2)

            # ---- phase C: fitness gather ----
            with tc.tile_pool(name="fit", bufs=2) as wp:
                def fit_body(i):
                    ga = wp.tile([128, D0P, 64], dt.float32, tag="ga")
                    nc.gpsimd.dma_gather(
                        out_ap=ga[:], in_ap=a_d[:, :],
                        idxs_ap=pidx_sb[:, ds(i * SPB, SPB)],
                        num_idxs=128 * D0P, num_idxs_reg=128 * D0P,
                        elem_size=64, single_packet=False)
                    zs = wp.tile([128, 1], dt.float32, tag="zs")
                    nc.vector.tensor_reduce(zs[:], ga[:, :, 0:1].squeeze(2),
                                            axis=mybir.AxisListType.X,
                                            op=mybir.AluOpType.add)
                    zb = wp.tile([128, 1], dt.float32, tag="zb2")
                    nc.sync.dma_start(zb[:], zb_d[ds(i * 128, 128), :])
                    nc.vector.tensor_add(zs[:], zs[:], zb[:])
                    nc.sync.dma_start(z_d[ds(i * 128, 128), :], zs[:])
                tc.For_i_unrolled(0, NB0, 1, fit_body, max_unroll=4)
    nc.compile()
    return nc


def _build_L12():
    """Layers 1/2: conv + pool + fitness over fixed-degree kNN graph."""
    B = _get_bass()
    bacc, mybir, TileContext = B["bacc"], B["mybir"], B["TileContext"]
    ds, make_identity = B["ds"], B["make_identity"]
    dt = mybir.dt
    F = HID
    SC = NB1 * ((128 * D1C) // 16)
    SP = NB1 * ((128 * D1P) // 16)
    HROWS = R1 + 128                    # sentinel = R1

    nc = bacc.Bacc("TRN2", target_bir_lowering=False)
    x_d = nc.dram_tensor("x", [XN_ROWS, F], dt.float32, kind="ExternalInput")
    xT_d = nc.dram_tensor("xT", [F, XT_COLS], dt.float32, kind="ExternalInput")
    cidx_d = nc.dram_tensor("cidx", [128, SC], dt.int16, kind="ExternalInput")
    pidx_d = nc.dram_tensor("pidx", [128, SP], dt.int16, kind="ExternalInput")
    invdeg_d = nc.dram_tensor("invdeg", [128, 1], dt.float32,
                              kind="ExternalInput")
    cnt_d = nc.dram_tensor("cnt", [128, 1], dt.float32, kind="ExternalInput")
    wr_d = nc.dram_tensor("wr", [128, 4, HID], dt.float32,
                          kind="ExternalInput")
    wl_d = nc.dram_tensor("wl", [128, 4, HID], dt.float32,
                          kind="ExternalInput")
    br_d = nc.dram_tensor("br", [128, HID], dt.float32, kind="ExternalInput")
    wq_d = nc.dram_tensor("wq", [128, HID], dt.float32, kind="ExternalInput")
    aw2_d = nc.dram_tensor("aw2", [128, HID], dt.float32, kind="ExternalInput")
    l1w_d = nc.dram_tensor("l1w", [128, HID], dt.float32, kind="ExternalInput")
    l2w_d = nc.dram_tensor("l2w", [128, HID], dt.float32, kind="ExternalInput")
    l3w_d = nc.dram_tensor("l3w", [128, HID], dt.float32, kind="ExternalInput")
    qb_d = nc.dram_tensor("qb", [128, 1], dt.float32, kind="ExternalInput")
    l1b_d = nc.dram_tensor("l1b", [128, 1], dt.float32, kind="ExternalInput")
    l3b_d = nc.dram_tensor("l3b", [128, 1], dt.float32, kind="ExternalInput")

    h_d = nc.dram_tensor("fh", [HROWS, 576], dt.float32, kind="Internal")
    a_d = nc.dram_tensor("fa", [HROWS, 64], dt.float32, kind="Internal")
    zb_d = nc.dram_tensor("zb", [R1, 1], dt.float32, kind="Internal")
    xn_d = nc.dram_tensor("xn", [XN_ROWS, HID], dt.float32,
                          kind="ExternalOutput")
    z_d = nc.dram_tensor("z", [R1, 1], dt.float32, kind="ExternalOutput")

    with TileContext(nc) as tc:
        with (
            tc.tile_pool(name="const", bufs=1) as cpool,
            tc.tile_pool(name="ps", bufs=2, space="PSUM") as pspool,
        ):
            ident = cpool.tile([128, 128], dt.float32)
            make_identity(nc, ident[:])
            wr_sb = cpool.tile([128, 4, HID], dt.float32)
            nc.sync.dma_start(wr_sb[:], wr_d[:, :, :])
            wl_sb = cpool.tile([128, 4, HID], dt.float32)
            nc.sync.dma_start(wl_sb[:], wl_d[:, :, :])
            br_sb = cpool.tile([128, HID], dt.float32)
            nc.sync.dma_start(br_sb[:], br_d[:, :])
            wq_sb = cpool.tile([128, HID], dt.float32)
            nc.sync.dma_start(wq_sb[:], wq_d[:, :])
            aw2_sb = cpool.tile([128, HID], dt.float32)
            nc.sync.dma_start(aw2_sb[:], aw2_d[:, :])
            l1w_sb = cpool.tile([128, HID], dt.float32)
            nc.sync.dma_start(l1w_sb[:], l1w_d[:, :])
            l2w_sb = cpool.tile([128, HID], dt.float32)
            nc.sync.dma_start(l2w_sb[:], l2w_d[:, :])
            l3w_sb = cpool.tile([128, HID], dt.float32)
            nc.sync.dma_start(l3w_sb[:], l3w_d[:, :])
            qb_sb = cpool.tile([128, 1], dt.float32)
            nc.sync.dma_start(qb_sb[:], qb_d[:, :])
            l1b_sb = cpool.tile([128, 1], dt.float32)
            nc.sync.dma_start(l1b_sb[:], l1b_d[:, :])
            l3b_sb = cpool.tile([128, 1], dt.float32)
            nc.sync.dma_start(l3b_sb[:], l3b_d[:, :])
            iv_sb = cpool.tile([128, 1], dt.float32)
            nc.sync.dma_start(iv_sb[:], invdeg_d[:, :])
            ct_sb = cpool.tile([128, 1], dt.float32)
            nc.sync.dma_start(ct_sb[:], cnt_d[:, :])
            cidx_sb = cpool.tile([128, SC], dt.int16)
            nc.sync.dma_start(cidx_sb[:], cidx_d[:, :])
            pidx_sb = cpool.tile([128, SP], dt.int16)
            nc.sync.dma_start(pidx_sb[:], pidx_d[:, :])
            srow = cpool.tile([1, 576], dt.float32)
            nc.vector.memset(srow[:], 0.0)
            nc.vector.memset(srow[:, 512:513], -1e30)
            nc.sync.dma_start(h_d[R1:R1 + 1, :], srow[:])
            nc.sync.dma_start(a_d[R1:R1 + 1, :], srow[:, 0:64])

            SCB = (128 * D1C) // 16
            SPB = (128 * D1P) // 16
            with tc.tile_pool(name="conv", bufs=2) as wp:
                def conv_body(i):
                    g = wp.tile([128, D1C, F], dt.float32, tag="g")
                    nc.gpsimd.dma_gather(
                        out_ap=g[:], in_ap=x_d[:, :],
                        idxs_ap=cidx_sb[:, ds(i * SCB, SCB)],
                        num_idxs=128 * D1C, num_idxs_reg=128 * D1C,
                        elem_size=F, single_packet=False)
                    _tree_sum(nc, g, D1C,
                              lambda lo, cnt: g[:, lo:lo + cnt, :])
                    mean = wp.tile([128, F], dt.float32, tag="mean")
                    nc.vector.tensor_scalar_mul(mean[:], g[:, 0, :], iv_sb[:])
                    hps = pspool.tile([128, HID], dt.float32, tag="hps")
                    xt = wp.tile([128, 4, 128], dt.float32, tag="xt")
                    nc.sync.dma_start(
                        xt[:], xT_d[:, ds(i * 128, 128)].rearrange(
                            "(c r) m -> r c m", c=4))
                    mt = wp.tile([128, 4, 128], dt.float32, tag="mt")
                    for c in range(4):
                        tp = pspool.tile([128, 128], dt.float32, tag="tp")
                        nc.tensor.transpose(tp[:],
                                            mean[:, c * 128:(c + 1) * 128],
                                            ident[:])
                        nc.vector.tensor_copy(mt[:, c, :], tp[:])
                    for c in range(4):
                        nc.tensor.matmul(hps[:], xt[:, c, :], wl_sb[:, c, :],
                                         start=(c == 0), stop=False)
                    for c in range(4):
                        nc.tensor.matmul(hps[:], mt[:, c, :], wr_sb[:, c, :],
                                         start=False, stop=(c == 3))
                    hsb = wp.tile([128, 576], dt.float32, tag="hsb")
                    nc.vector.tensor_add(
                        hsb[:, 0:HID], hps[:],
                        br_sb[:])
                    nc.vector.tensor_scalar_max(hsb[:, 0:HID], hsb[:, 0:HID],
                                                0.0)
                    tmp = wp.tile([128, HID], dt.float32, tag="tmp")
                    nc.vector.tensor_mul(tmp[:], hsb[:, 0:HID],
                                         aw2_sb[:])
                    nc.vector.tensor_reduce(hsb[:, 512:513], tmp[:],
                                            axis=mybir.AxisListType.X,
                                            op=mybir.AluOpType.add)
                    nc.sync.dma_start(h_d[ds(i * 128, 128), 0:513],
                                      hsb[:, 0:513])
                tc.For_i_unrolled(0, NB1, 1, conv_body, max_unroll=2)

            with tc.tile_pool(name="pool", bufs=2) as wp:
                def pool_body(i):
                    g = wp.tile([128, D1P, 576], dt.float32, tag="g")
                    nc.gpsimd.dma_gather(
                        out_ap=g[:], in_ap=h_d[:, :],
                        idxs_ap=pidx_sb[:, ds(i * SPB, SPB)],
                        num_idxs=128 * D1P, num_idxs_reg=128 * D1P,
                        elem_size=576, single_packet=False)
                    xq = wp.tile([128, D1P // 2, HID], dt.float32, tag="xq")
                    _tree_max(nc, xq, g, D1P,
                              lambda lo, cnt: g[:, lo:lo + cnt, 0:HID],
                              lambda lo, cnt: xq[:, lo:lo + cnt, :])
                    tmp = wp.tile([128, HID], dt.float32, tag="tmp")
                    nc.vector.tensor_mul(tmp[:], xq[:, 0, :],
                                         wq_sb[:])
                    qs = wp.tile([128, 1], dt.float32, tag="qs")
                    nc.vector.tensor_reduce(qs[:], tmp[:],
                                            axis=mybir.AxisListType.X,
                                            op=mybir.AluOpType.add)
                    nc.vector.tensor_add(qs[:], qs[:], qb_sb[:])
                    sc = wp.tile([128, D1P], dt.float32, tag="sc")
                    jsv = g[:, :, 512:513].squeeze(2)
                    nc.vector.tensor_scalar_add(sc[:], jsv, qs[:])
                    sc2 = wp.tile([128, D1P], dt.float32, tag="sc2")
                    nc.vector.tensor_scalar_mul(sc2[:], sc[:], 0.2)
                    nc.vector.tensor_max(sc[:], sc[:], sc2[:])
                    m = wp.tile([128, 1], dt.float32, tag="m")
                    nc.vector.tensor_reduce(m[:], sc[:],
                                            axis=mybir.AxisListType.X,
                                            op=mybir.AluOpType.max)
                    nc.vector.tensor_scalar(sc[:], sc[:], m[:], None,
                                            op0=mybir.AluOpType.subtract)
                    nc.scalar.activation(sc[:], sc[:],
                                         mybir.ActivationFunctionType.Exp)
                    ssum = wp.tile([128, 1], dt.float32, tag="ssum")
                    nc.vector.tensor_reduce(ssum[:], sc[:],
                                            axis=mybir.AxisListType.X,
                                            op=mybir.AluOpType.add)
                    rec = wp.tile([128, 1], dt.float32, tag="rec")
                    nc.vector.reciprocal(rec[:], ssum[:])
                    nc.vector.tensor_scalar_mul(sc[:], sc[:], rec[:])
                    gh = g[:, :, 0:HID]
                    nc.vector.tensor_mul(
                        gh, gh, sc[:].unsqueeze(2).to_broadcast(
                            [128, D1P, HID]))
                    _tree_sum(nc, g, D1P,
                              lambda lo, cnt: g[:, lo:lo + cnt, 0:HID])
                    xn = g[:, 0, 0:HID]
                    nc.sync.dma_start(xn_d[ds(i * 128, 128), :], xn)
                    nc.vector.tensor_mul(tmp[:], xn,
                                         l1w_sb[:])
                    av = wp.tile([128, 1], dt.float32, tag="av")
                    nc.vector.tensor_reduce(av[:], tmp[:],
                                            axis=mybir.AxisListType.X,
                                            op=mybir.AluOpType.add)
                    nc.sync.dma_start(a_d[ds(i * 128, 128), 0:1], av[:])
                    nc.vector.tensor_mul(tmp[:], xn,
                                         l2w_sb[:])
                    bv = wp.tile([128, 1], dt.float32, tag="bv")
                    nc.vector.tensor_reduce(bv[:], tmp[:],
                                            axis=mybir.AxisListType.X,
                                            op=mybir.AluOpType.add)
                    nc.vector.tensor_mul(tmp[:], xn,
                                         l3w_sb[:])
                    cv = wp.tile([128, 1], dt.float32, tag="cv")
                    nc.vector.tensor_reduce(cv[:], tmp[:],
                                            axis=mybir.AxisListType.X,
                                            op=mybir.AluOpType.add)
                    zb = wp.tile([128, 1], dt.float32, tag="zb")
                    nc.vector.tensor_mul(zb[:], ct_sb[:], bv[:])
                    nc.vector.tensor_sub(zb[:], cv[:], zb[:])
                    nc.vector.tensor_add(zb[:], zb[:], l3b_sb[:])
                    lb1 = wp.tile([128, 1], dt.float32, tag="lb1")
                    nc.vector.tensor_mul(lb1[:], ct_sb[:], l1b_sb[:])
                    nc.vector.tensor_add(zb[:], zb[:], lb1[:])
                    nc.sync.dma_start(zb_d[ds(i * 128, 128), :], zb[:])
                tc.For_i_unrolled(0, NB1, 1, pool_body, max_unroll=2)

            with tc.tile_pool(name="fit", bufs=2) as wp:
                def fit_body(i):
                    ga = wp.tile([128, D1P, 64], dt.float32, tag="ga")
                    nc.gpsimd.dma_gather(
                        out_ap=ga[:], in_ap=a_d[:, :],
                        idxs_ap=pidx_sb[:, ds(i * SPB, SPB)],
                        num_idxs=128 * D1P, num_idxs_reg=128 * D1P,
                        elem_size=64, single_packet=False)
                    zs = wp.tile([128, 1], dt.float32, tag="zs")
                    nc.vector.tensor_reduce(zs[:], ga[:, :, 0:1].squeeze(2),
                                            axis=mybir.AxisListType.X,
                                            op=mybir.AluOpType.add)
                    zb = wp.tile([128, 1], dt.float32, tag="zb2")
                    nc.sync.dma_start(zb[:], zb_d[ds(i * 128, 128), :])
                    nc.vector.tensor_add(zs[:], zs[:], zb[:])
                    nc.sync.dma_start(z_d[ds(i * 128, 128), :], zs[:])
                tc.For_i_unrolled(0, NB1, 1, fit_body, max_unroll=4)
    nc.compile()
    return nc


def _build_K():
    """Select (gather xn[perm]*fv -> x, xT, running max) + kNN scan."""
    B = _get_bass()
    bacc, mybir, TileContext = B["bacc"], B["mybir"], B["TileContext"]
    ds, make_identity = B["ds"], B["make_identity"]
    dt = mybir.dt
    SS = NB1 * ((128 * 1) // 16)   # select idx cols (1 slot per row)

    nc = bacc.Bacc("TRN2", target_bir_lowering=False)
    xn_d = nc.dram_tensor("xn", [XN_ROWS, HID], dt.float32,
                          kind="ExternalInput")
    sidx_d = nc.dram_tensor("sidx", [128, SS], dt.int16, kind="ExternalInput")
    fv_d = nc.dram_tensor("fv", [R1, 1], dt.float32, kind="ExternalInput")
    msk_d = nc.dram_tensor("msk", [R1, 1], dt.float32, kind="ExternalInput")
    qT_d = nc.dram_tensor("qT", [4, XT_COLS], dt.float32, kind="ExternalInput")
    cand_d = nc.dram_tensor("cand", [4, XT_COLS], dt.float32,
                            kind="ExternalInput")
    x_d = nc.dram_tensor("xo", [XN_ROWS, HID], dt.float32,
                         kind="ExternalOutput")
    xT_d = nc.dram_tensor("xT", [HID, XT_COLS], dt.float32,
                          kind="ExternalOutput")
    xsp_d = nc.dram_tensor("xsp", [128, HID], dt.float32,
                           kind="ExternalOutput")
    knn_d = nc.dram_tensor("knn", [R1, 16], dt.uint16, kind="ExternalOutput")

    with TileContext(nc) as tc:
        with (
            tc.tile_pool(name="const", bufs=1) as cpool,
            tc.tile_pool(name="ps", bufs=2, space="PSUM") as pspool,
        ):
            ident = cpool.tile([128, 128], dt.float32)
            make_identity(nc, ident[:])
            sidx_sb = cpool.tile([128, SS], dt.int16)
            nc.sync.dma_start(sidx_sb[:], sidx_d[:, :])
            cand_sb = cpool.tile([4, XT_COLS], dt.float32)
            nc.sync.dma_start(cand_sb[:], cand_d[:, :])
            runmax = cpool.tile([128, HID], dt.float32)
            nc.vector.memset(runmax[:], -1e30)

            SSB = 8   # (128*1)//16
            with tc.tile_pool(name="sel", bufs=2) as wp:
                def sel_body(i):
                    g = wp.tile([128, 1, HID], dt.float32, tag="g")
                    nc.gpsimd.dma_gather(
                        out_ap=g[:], in_ap=xn_d[:, :],
                        idxs_ap=sidx_sb[:, ds(i * SSB, SSB)],
                        num_idxs=128, num_idxs_reg=128,
                        elem_size=HID, single_packet=False)
                    fv = wp.tile([128, 1], dt.float32, tag="fv")
                    nc.sync.dma_start(fv[:], fv_d[ds(i * 128, 128), :])
                    xs = wp.tile([128, HID], dt.float32, tag="xs")
                    nc.vector.tensor_scalar_mul(xs[:], g[:, 0, :], fv[:])
                    nc.sync.dma_start(x_d[ds(i * 128, 128), :], xs[:])
                    mk = wp.tile([128, 1], dt.float32, tag="mk")
                    nc.sync.dma_start(mk[:], msk_d[ds(i * 128, 128), :])
                    xm = wp.tile([128, HID], dt.float32, tag="xm2")
                    nc.vector.tensor_scalar_add(xm[:], xs[:], mk[:])
                    nc.vector.tensor_max(runmax[:], runmax[:], xm[:])
                    for c in range(4):
                        tp = pspool.tile([128, 128], dt.float32, tag="tp")
                        nc.tensor.transpose(tp[:],
                                            xs[:, c * 128:(c + 1) * 128],
                                            ident[:])
                        tt = wp.tile([128, 128], dt.float32, tag="tt")
                        nc.vector.tensor_copy(tt[:], tp[:])
                        nc.sync.dma_start(
                            xT_d[c * 128:(c + 1) * 128, ds(i * 128, 128)],
                            tt[:])
                tc.For_i_unrolled(0, NB1, 1, sel_body, max_unroll=2)
            nc.sync.dma_start(xsp_d[:, :], runmax[:])

            with tc.tile_pool(name="knn", bufs=2) as wp:
                def knn_body(i):
                    qsb = wp.tile([4, 128], dt.float32, tag="q")
                    nc.sync.dma_start(qsb[:], qT_d[:, ds(i * 128, 128)])
                    row = wp.tile([128, XT_COLS], dt.float32, tag="row")
                    for ch in range(NCH):
                        dps = pspool.tile([128, 512], dt.float32, tag="d")
                        nc.tensor.matmul(dps[:], qsb[:],
                                         cand_sb[:, ch * 512:(ch + 1) * 512],
                                         start=True, stop=True)
                        nc.scalar.activation(
                            row[:, ch * 512:(ch + 1) * 512], dps[:],
                            mybir.ActivationFunctionType.Copy)
                    v8 = wp.tile([128, 8], dt.float32, tag="v8")
                    nc.vector.max(out=v8[:], in_=row[:])
                    i16 = wp.tile([128, 16], dt.uint16, tag="i16")
                    i8 = wp.tile([128, 8], dt.uint32, tag="i8")
                    nc.vector.max_index(i8[:], v8[:], row[:])
                    nc.vector.tensor_copy(i16[:, 0:8], i8[:])
                    nc.vector.match_replace(out=row[:], in_to_replace=v8[:],
                                            in_values=row[:], imm_value=-3e30)
                    v8b = wp.tile([128, 8], dt.float32, tag="v8b")
                    nc.vector.max(out=v8b[:], in_=row[:])
                    i8b = wp.tile([128, 8], dt.uint32, tag="i8b")
                    nc.vector.max_index(i8b[:], v8b[:], row[:])
                    nc.vector.tensor_copy(i16[:, 8:16], i8b[:])
                    nc.sync.dma_start(knn_d[ds(i * 128, 128), :], i16[:])
                tc.For_i_unrolled(0, NB1, 1, knn_body, max_unroll=2)
    nc.compile()
    return nc


# ----------------------------------------------------------------------------
# build/compile management (import-time warm-up)
# ----------------------------------------------------------------------------

_RUNNERS = {}
_BUILD_LOCK = threading.Lock()
_BUILD_THREADS = []


def _get_runner(name, builder):
    with _BUILD_LOCK:
        if name in _RUNNERS:
            return _RUNNERS[name]
    r = _Launcher(builder()).warm()
    with _BUILD_LOCK:
        _RUNNERS.setdefault(name, r)
    return _RUNNERS[name]


def _warm():
    try:
        _get_bass()
        ncs = {}
        ncs["L0_%d" % D0C_DEFAULT] = _build_L0(D0C_DEFAULT)
        ncs["L12"] = _build_L12()
        ncs["K"] = _build_K()
        launchers = {name: _Launcher(nc) for name, nc in ncs.items()}
        # compile the three programs in parallel (neuronx-cc subprocesses)
        ths = [threading.Thread(target=l.warm) for l in launchers.values()]
        for t in ths:
            t.start()
        for t in ths:
            t.join()
        with _BUILD_LOCK:
            for name, l in launchers.items():
                _RUNNERS.setdefault(name, l)
    except Exception:  # pragma: no cover - fallback path handles
        import traceback
        traceback.print_exc()


_BUILD_THREADS.append(threading.Thread(target=_warm, daemon=True))
_BUILD_THREADS[-1].start()


# ----------------------------------------------------------------------------
# numpy fallbacks (used only if the device path fails)
# ----------------------------------------------------------------------------

def _np_reference(x, pos, src, dst, W):
    f = _f32
    n = N0
    xs = []
    for i in range(L):
        wr, br, wl = W["wr"][i], W["br"][i], W["wl"][i]
        agg = np.zeros((n, x.shape[1]), f)
        np.add.at(agg, dst, x[src])
        deg = np.bincount(dst, minlength=n).astype(f)
        mean = agg / np.maximum(deg, 1)[:, None]
        h = np.maximum(mean @ wr + br + x @ wl, 0).astype(f)
        sl = np.arange(n)
        s_ = np.concatenate([src, sl])
        d_ = np.concatenate([dst, sl])
        xj = h[s_]
        xq = np.full((n, HID), -np.inf, f)
        np.maximum.at(xq, d_, xj)
        xq = (xq @ W["lw"][i] + W["lb"][i]).astype(f)
        aw, ab = W["aw"][i], W["ab"][i]
        score = (xq[d_] @ aw[:HID] + xj @ aw[HID:] + ab).astype(f)
        score = np.where(score > 0, score, f(0.2) * score).astype(f)
        smax = np.full(n, -np.inf, f)
        np.maximum.at(smax, d_, score)
        ex = np.exp(score - smax[d_])
        ssum = np.zeros(n, f)
        np.add.at(ssum, d_, ex)
        att = (ex / ssum[d_]).astype(f)
        xn = np.zeros((n, HID), f)
        np.add.at(xn, d_, xj * att[:, None])
        a = xn @ W["l1w"][i] + W["l1b"][i]
        b = xn @ W["l2w"][i]
        agg2 = np.zeros(n, f)
        np.add.at(agg2, d_, (a[s_] - b[d_]).astype(f))
        z = (agg2 + xn @ W["l3w"][i] + W["l3b"][i]).astype(f)
        k_keep = int(math.ceil(RATIO * n))
        fit64 = 1.0 / (1.0 + np.exp(-z.astype(np.float64)))
        perm = np.argpartition(-fit64, k_keep - 1)[:k_keep]
        fv = fit64[perm].astype(f)
        x = (xn[perm] * fv[:, None]).astype(f)
        xs.append(x.max(0))
        pos = pos[perm]
        n = k_keep
        if i < L - 1:
            k = 6 + 2 * i
            sq = np.sum(pos * pos, -1)
            dist = sq[:, None] + sq[None, :] - 2 * (pos @ pos.T)
            np.fill_diagonal(dist, np.inf)
            idx = np.argpartition(dist, k, 1)[:, :k]
            srt = np.take_along_axis(dist, idx, 1).argsort(1, kind="stable")
            idx = np.take_along_axis(idx, srt, 1)
            dst = np.repeat(np.arange(n), k)
            src = idx.reshape(-1)
    return xs


# ----------------------------------------------------------------------------
# kNN host validation
# ----------------------------------------------------------------------------

def _knn_from_cand(cand16, pos, k):
    """cand16: [n, 16] device max-index results (cols sorted by -dist).
    Returns tbl [n, k] of neighbor ids; falls back per-row when needed."""
    n = pos.shape[0]
    selfid = np.arange(n, dtype=np.int64)
    c = cand16.astype(np.int64)
    not_self = c != selfid[:, None]
    # positions of first k non-self entries per row
    cum = np.cumsum(not_self, 1)
    takec = (cum <= k) & not_self
    enough = cum[:, -1] >= k
    tbl = np.zeros((n, k), np.int64)
    rows_ok = np.flatnonzero(enough)
    # fill via argsort trick: order of selected cols preserved
    sel = np.where(takec, np.arange(16)[None, :], 99)
    ordcols = np.argsort(sel, 1, kind="stable")[:, :k]
    tbl = np.take_along_axis(c, ordcols, 1)
    # validity: unique and in range
    srt = np.sort(tbl, 1)
    dup = (srt[:, 1:] == srt[:, :-1]).any(1)
    oob = (tbl < 0).any(1) | (tbl >= n).any(1)
    bad = dup | oob | ~enough
    bad_rows = np.flatnonzero(bad)
    if len(bad_rows):
        sq = np.sum(pos * pos, 1)
        for i in bad_rows:
            d = sq + sq[i] - 2.0 * (pos @ pos[i])
            d[i] = np.inf
            idx = np.argpartition(d, k)[:k]
            tbl[i] = idx[np.argsort(d[idx], kind="stable")]
    return tbl


# ----------------------------------------------------------------------------
# main kernel
# ----------------------------------------------------------------------------

_EXEC_NS = []


def kernel(x, pos, edge_index, conv0_wr, conv0_br, conv0_wl, conv_wr, conv_br,
           conv_wl, pool_lin_w, pool_lin_b, pool_att_w, pool_att_b, le1_w,
           le1_b, le2_w, le3_w, le3_b, lin1_w, lin1_b, lin2_w, lin2_b):
    t_start = time.perf_counter()
    _EXEC_NS.clear()
    x = np.asarray(x, _f32)
    pos = np.asarray(pos, _f32)
    ei = np.asarray(edge_index).astype(np.int64)

    W = {
        "wr": [np.asarray(conv0_wr, _f32)] + [np.asarray(conv_wr[i], _f32)
                                              for i in range(L - 1)],
        "br": [np.asarray(conv0_br, _f32)] + [np.asarray(conv_br[i], _f32)
                                              for i in range(L - 1)],
        "wl": [np.asarray(conv0_wl, _f32)] + [np.asarray(conv_wl[i], _f32)
                                              for i in range(L - 1)],
        "lw": [np.asarray(pool_lin_w[i], _f32) for i in range(L)],
        "lb": [np.asarray(pool_lin_b[i], _f32) for i in range(L)],
        "aw": [np.asarray(pool_att_w[i], _f32) for i in range(L)],
        "ab": [float(pool_att_b[i]) for i in range(L)],
        "l1w": [np.asarray(le1_w[i], _f32) for i in range(L)],
        "l1b": [float(le1_b[i]) for i in range(L)],
        "l2w": [np.asarray(le2_w[i], _f32) for i in range(L)],
        "l3w": [np.asarray(le3_w[i], _f32) for i in range(L)],
        "l3b": [float(le3_b[i]) for i in range(L)],
    }
    try:
        xs = _device_forward(x, pos, ei, W)
    except Exception:
        import traceback
        traceback.print_exc()
        print("kernel: device path failed; numpy fallback")
        xs = _np_reference(x, pos, ei[0], ei[1], W)

    hcat = np.concatenate(xs)[None, :].astype(_f32)
    h1 = np.maximum(hcat @ np.asarray(lin1_w, _f32) +
                    np.asarray(lin1_b, _f32), 0)
    out = (h1 @ np.asarray(lin2_w, _f32) + np.asarray(lin2_b, _f32))
    dt_ns = int((time.perf_counter() - t_start) * 1e9)
    _EXEC_NS.append(("kernel", dt_ns))
    return out.astype(_f32)


def _layer_weights(W, i):
    """Pack per-layer pool/fitness weight vectors for the L programs."""
    lw, lb = W["lw"][i], W["lb"][i]
    aw, ab = W["aw"][i], W["ab"][i]
    wq = (lw @ aw[:HID]).astype(_f32)
    qb = float(lb @ aw[:HID] + ab)
    rep = lambda v: np.ascontiguousarray(
        np.broadcast_to(np.asarray(v, _f32), (128, HID)))
    return {
        "br": rep(W["br"][i]),
        "wq": rep(wq),
        "aw2": rep(aw[HID:]),
        "l1w": rep(W["l1w"][i]),
        "l2w": rep(W["l2w"][i]),
        "l3w": rep(W["l3w"][i]),
        "qb": _rep128(qb),
        "l1b": _rep128(W["l1b"][i]),
        "l3b": _rep128(W["l3b"][i]),
    }


def _device_forward(x, pos, ei, W):
    B = _get_bass()
    jax, jnp = B["jax"], B["jnp"]
    dev = jax.devices()[0]
    put = lambda a: jax.device_put(a, dev)
    src, dst = ei[0], ei[1]

    # ---------------- layer 0 host prep ----------------
    deg0 = np.bincount(dst, minlength=R0).astype(np.int64)
    D0C = max(int(deg0.max()), 1)
    name0 = "L0_%d" % D0C
    # kick off / reuse builds
    for th in _BUILD_THREADS:
        th.join()
    L0run = _RUNNERS.get(name0) or _get_runner(name0, lambda: _build_L0(D0C))
    L12run = _RUNNERS.get("L12") or _get_runner("L12", _build_L12)
    Krun = _RUNNERS.get("K") or _get_runner("K", _build_K)

    x0 = np.zeros((X0_ROWS, IN_CH), _f32)
    x0[:N0] = x
    x0_dev = put(x0)

    SENT0 = R0
    tblC, _ = _slot_table(src, dst, R0, D0C, SENT0)
    cidx0 = _idx_to_i16_tile(_slotmajor_list(tblC))
    tblP = np.concatenate(
        [np.arange(R0, dtype=np.int64)[:, None], tblC], 1)
    tblP[N0:, 0] = SENT0   # pad rows: no self slot
    pidx0 = _idx_to_i16_tile(_slotmajor_list(tblP))
    invdeg0 = (1.0 / np.maximum(deg0, 1.0)).astype(_f32)[:, None]
    cnt0 = (deg0 + 1).astype(_f32)[:, None]
    lw0 = _layer_weights(W, 0)
    wxm = np.zeros((128, HID), _f32)
    wxm[0:IN_CH] = W["wl"][0]
    wxm[IN_CH:2 * IN_CH] = W["wr"][0]

    in0 = {"x": x0_dev, "cidx": put(cidx0), "pidx": put(pidx0),
           "invdeg": put(invdeg0), "cnt": put(cnt0), "wxm": put(wxm)}
    in0.update({k: put(v) for k, v in lw0.items()})
    t0 = time.perf_counter()
    r0 = L0run(in0)
    z0 = np.asarray(r0["z"])[:N0, 0]
    _EXEC_NS.append(("L0", int((time.perf_counter() - t0) * 1e9)))

    xs_out = []
    feat_xn = r0["xn"]
    cur_pos = pos
    n_cur = N0
    for i in range(L):
        k_keep = int(math.ceil(RATIO * n_cur))
        z = z0
        # ---- host top-k ----
        perm = np.argpartition(-z, k_keep - 1)[:k_keep]
        fit = (1.0 / (1.0 + np.exp(-z[perm].astype(np.float64)))).astype(_f32)
        sel = np.zeros(R1, np.int64)
        sel[:k_keep] = perm
        fv = np.zeros((R1, 1), _f32)
        fv[:k_keep, 0] = fit
        msk = np.full((R1, 1), -1e30, _f32)
        msk[:k_keep] = 0.0
        cur_pos = cur_pos[perm]
        n_cur = k_keep
        # ---- kNN inputs ----
        if i < L - 1:
            kk = 6 + 2 * i
            sq = np.sum(cur_pos * cur_pos, 1, dtype=_f32)
            qT = np.zeros((4, XT_COLS), _f32)
            qT[0, :n_cur] = 2.0 * cur_pos[:, 0]
            qT[1, :n_cur] = 2.0 * cur_pos[:, 1]
            qT[2, :n_cur] = -1.0
            qT[3, :n_cur] = -sq
            cand = np.zeros((4, XT_COLS), _f32)
            cand[0, :n_cur] = cur_pos[:, 0]
            cand[1, :n_cur] = cur_pos[:, 1]
            cand[2, :n_cur] = sq
            cand[2, n_cur:] = 1e30
            cand[3, :] = 1.0
        else:
            kk = 0
            qT = np.zeros((4, XT_COLS), _f32)
            cand = np.zeros((4, XT_COLS), _f32)
        t0 = time.perf_counter()
        rK = Krun({"xn": feat_xn, "sidx": put(_idx_to_i16_tile(sel)),
                   "fv": put(fv), "msk": put(msk),
                   "qT": put(qT), "cand": put(cand)})
        xsp = np.asarray(rK["xsp"])
        xs_out.append(xsp.max(0))
        _EXEC_NS.append(("K%d" % i, int((time.perf_counter() - t0) * 1e9)))
        if i == L - 1:
            break
        cand16 = np.asarray(rK["knn"])[:n_cur]
        tbl = _knn_from_cand(cand16, cur_pos, kk)

        # ---- next layer tables ----
        SENT1 = R1
        tblC1 = np.full((R1, D1C), SENT1, np.int64)
        tblC1[:n_cur, :kk] = tbl
        cidx1 = _idx_to_i16_tile(_slotmajor_list(tblC1))
        tblP1 = np.concatenate(
            [np.arange(R1, dtype=np.int64)[:, None], tblC1], 1)
        tblP1[n_cur:, 0] = SENT1
        pidx1 = _idx_to_i16_tile(_slotmajor_list(tblP1))
        lwi = _layer_weights(W, i + 1)
        wr_c = np.ascontiguousarray(
            W["wr"][i + 1].reshape(4, 128, HID).transpose(1, 0, 2))
        wl_c = np.ascontiguousarray(
            W["wl"][i + 1].reshape(4, 128, HID).transpose(1, 0, 2))
        inL = {"x": rK["xo"], "xT": rK["xT"],
               "cidx": put(cidx1), "pidx": put(pidx1),
               "invdeg": _rep128(1.0 / kk), "cnt": _rep128(kk + 1),
               "wr": put(wr_c), "wl": put(wl_c)}
        inL["invdeg"] = put(inL["invdeg"])
        inL["cnt"] = put(inL["cnt"])
        inL.update({k: put(v) for k, v in lwi.items()})
        t0 = time.perf_counter()
        rL = L12run(inL)
        z0 = np.asarray(rL["z"])[:n_cur, 0]
        _EXEC_NS.append(("L%d" % (i + 1),
                         int((time.perf_counter() - t0) * 1e9)))
        feat_xn = rL["xn"]
    return xs_out


def total_exec_ns():
    return sum(v for k, v in _EXEC_NS if k == "kernel")


def exec_breakdown():
    return list(_EXEC_NS)


# revision 8
# speedup vs baseline: 7.2565x; 4.7162x over previous
"""ASAP-GNN classifier on trn2 via Bass/Tile.

Architecture (v2): single NeuronCore, device-resident features between
launches. Three compiled programs (NEFFs), built/compiled at import time in
background threads:

  L0  : layer-0 GraphConv + ASAPool attention + LEConv fitness over the
        irregular input graph (slot-table gathers, For_i loops over 157
        row-blocks of 128 nodes).
  L12 : same pipeline for layers 1 and 2 over the fixed-degree kNN graphs
        (shared program; layer-2's 5000 nodes padded to layer-1's shape).
  K   : top-half "select" (gather xn[perm]*fv -> next x + transposed copy +
        running global max) fused with the dense kNN distance scan
        (max8/max_index, two rounds -> 16 neighbor candidates).

Host does only: slot-table construction, top-k via argpartition on the
fitness logits, kNN candidate validation, and the final 1x1536 MLP. Per
layer one launch round-trip for fitness -> perm and one for select+kNN:
6 launches total, ~KBs of traffic each after the initial ~17MB upload.
"""

import math
import threading
import time
import numpy as np

N0 = 20000
IN_CH = 64
HID = 512
OUT = 10
L = 3
RATIO = 0.5

_f32 = np.float32

# ---- geometry constants (hardcoded; program shapes) ----
NB0 = 157                   # layer-0 row blocks
R0 = NB0 * 128              # 20096
X0_ROWS = R0 + 128          # feat_x0 rows (sentinel row = R0, zeros)
D0C_DEFAULT = 17            # layer-0 max in-degree (rebuilt if actual differs)

NB1 = 79                    # layer-1/2 row blocks
R1 = NB1 * 128              # 10112
D1C = 8                     # conv slots for kNN layers (k<=8)
D1P = 9                     # pool slots (self + 8)

XN_ROWS = 20352             # unified xn/x buffer rows (>= R0 + sentinel)
XT_COLS = 10240             # x1T columns (>= R1)
NCH = XT_COLS // 512        # kNN candidate chunks (20)


# ----------------------------------------------------------------------------
# bass plumbing
# ----------------------------------------------------------------------------

_BASS = {}


def _get_bass():
    if not _BASS:
        import concourse.bass as bass
        import concourse.bacc as bacc
        import concourse.mybir as mybir
        from concourse.tile import TileContext
        from concourse.masks import make_identity
        from concourse.bass import ds
        from concourse import bass2jax
        import jax
        import jax.numpy as jnp
        bass2jax.install_neuronx_cc_hook()
        _BASS.update(bass=bass, bacc=bacc, mybir=mybir, TileContext=TileContext,
                     make_identity=make_identity, ds=ds, bass2jax=bass2jax,
                     jax=jax, jnp=jnp)
    return _BASS


class _Launcher:
    """Compiled 1-core bass program; inputs/outputs stay jax device arrays."""

    def __init__(self, nc):
        B = _get_bass()
        jax, jnp, mybir = B["jax"], B["jnp"], B["mybir"]
        bass2jax = B["bass2jax"]
        partition_name = (nc.partition_id_tensor.name
                          if nc.partition_id_tensor else None)
        in_names, in_avals, out_names, out_avals = [], [], [], []
        for alloc in nc.m.functions[0].allocations:
            if not isinstance(alloc, mybir.MemoryLocationSet):
                continue
            name = alloc.memorylocations[0].name
            if alloc.kind == "ExternalInput":
                if name != partition_name:
                    in_names.append(name)
                    in_avals.append(jax.ShapeDtypeStruct(
                        tuple(alloc.tensor_shape), mybir.dt.np(alloc.dtype)))
            elif alloc.kind == "ExternalOutput":
                out_names.append(name)
                out_avals.append(jax.core.ShapedArray(
                    tuple(alloc.tensor_shape), mybir.dt.np(alloc.dtype)))
        self.in_names = in_names
        self.in_avals = in_avals
        self.out_names = out_names
        self.out_avals = out_avals
        n_params = len(in_names)
        all_names = in_names + out_names + (
            [partition_name] if partition_name else [])
        donate = tuple(range(n_params, n_params + len(out_names)))

        def _body(*args):
            operands = list(args)
            if partition_name is not None:
                operands.append(bass2jax.partition_id_tensor())
            outs = bass2jax._bass_exec_p.bind(
                *operands, out_avals=tuple(out_avals),
                in_names=tuple(all_names), out_names=tuple(out_names),
                lowering_input_output_aliases=(),
                sim_require_finite=True, sim_require_nnan=True, nc=nc)
            return tuple(outs)

        self._jit = jax.jit(_body, donate_argnums=donate, keep_unused=True)
        self._compiled = None

    def warm(self):
        """AOT-compile the executable (no execution)."""
        B = _get_bass()
        jax = B["jax"]
        out_structs = [jax.ShapeDtypeStruct(av.shape, av.dtype)
                       for av in self.out_avals]
        self._compiled = self._jit.lower(*self.in_avals,
                                         *out_structs).compile()
        return self

    def __call__(self, in_map):
        B = _get_bass()
        jnp = B["jnp"]
        args = [in_map[nm] for nm in self.in_names]
        zeros = [jnp.zeros(av.shape, av.dtype) for av in self.out_avals]
        fn = self._compiled if self._compiled is not None else self._jit
        outs = fn(*args, *zeros)
        return dict(zip(self.out_names, outs))


# ----------------------------------------------------------------------------
# host helpers
# ----------------------------------------------------------------------------

def _idx_to_i16_tile(idx_list):
    """dma_gather idx layout: element m -> partition m%16, col m//16,
    replicated across the 8 Q7 groups."""
    n = len(idx_list)
    S = (n + 15) // 16
    a = np.full((S, 16), -1, np.int16)
    a.reshape(-1)[:n] = idx_list.astype(np.int16)
    return np.ascontiguousarray(np.tile(a.T, (8, 1)))


def _slot_table(src, dst, nrows, D, sentinel):
    """[nrows, D] slot table: row i lists srcs of i's in-edges, sentinel pad."""
    deg = np.bincount(dst, minlength=nrows).astype(np.int64)
    order = np.argsort(dst, kind="stable")
    ss = src[order]
    dsrt = dst[order]
    starts = np.zeros(nrows + 1, np.int64)
    np.cumsum(deg, out=starts[1:])
    slot = np.arange(len(dsrt)) - starts[dsrt]
    tbl = np.full((nrows, D), sentinel, np.int64)
    tbl[dsrt, slot] = ss
    return tbl, deg


def _slotmajor_list(tbl):
    """[rows, D] -> block-slot-major gather list (per 128-block, slot-major)."""
    rows, D = tbl.shape
    nb = rows // 128
    return np.ascontiguousarray(
        tbl.reshape(nb, 128, D).transpose(0, 2, 1)).reshape(-1)


def _rep128(v):
    return np.full((128, 1), v, _f32)


# ----------------------------------------------------------------------------
# program builders
# ----------------------------------------------------------------------------

def _tree_sum(nc, g, n, view):
    """In-place binary-tree reduce over slot axis: view(g, lo, cnt) -> AP.
    Result lands in slot 0. Returns nothing."""
    w = n
    while w > 1:
        h = w // 2
        nc.vector.tensor_add(view(0, h), view(0, h), view(h, h))
        if w % 2:
            nc.vector.tensor_add(view(0, 1), view(0, 1), view(w - 1, 1))
        w = h


def _tree_max(nc, out_t, g, n, gview, oview):
    """Max over n slots of g into out_t (slot tile of n//2 width)."""
    h = n // 2
    nc.vector.tensor_max(oview(0, h), gview(0, h), gview(h, h))
    if n % 2:
        nc.vector.tensor_max(oview(0, 1), oview(0, 1), gview(n - 1, 1))
    w = h
    while w > 1:
        h2 = w // 2
        nc.vector.tensor_max(oview(0, h2), oview(0, h2), oview(h2, h2))
        if w % 2:
            nc.vector.tensor_max(oview(0, 1), oview(0, 1), oview(w - 1, 1))
        w = h2


def _build_L0(D0C):
    """Layer-0: conv + pool + fitness over irregular graph."""
    B = _get_bass()
    bacc, mybir, TileContext = B["bacc"], B["mybir"], B["TileContext"]
    ds, make_identity = B["ds"], B["make_identity"]
    dt = mybir.dt
    D0P = D0C + 1
    SC = NB0 * ((128 * D0C) // 16)       # conv idx tile cols
    SP = NB0 * ((128 * D0P) // 16)       # pool idx tile cols
    F = IN_CH
    HROWS = R0 + 128                     # feat_h rows, sentinel = R0

    nc = bacc.Bacc("TRN2", target_bir_lowering=False)
    x_d = nc.dram_tensor("x", [X0_ROWS, F], dt.float32, kind="ExternalInput")
    cidx_d = nc.dram_tensor("cidx", [128, SC], dt.int16, kind="ExternalInput")
    pidx_d = nc.dram_tensor("pidx", [128, SP], dt.int16, kind="ExternalInput")
    invdeg_d = nc.dram_tensor("invdeg", [R0, 1], dt.float32, kind="ExternalInput")
    cnt_d = nc.dram_tensor("cnt", [R0, 1], dt.float32, kind="ExternalInput")
    wxm_d = nc.dram_tensor("wxm", [128, HID], dt.float32, kind="ExternalInput")
    br_d = nc.dram_tensor("br", [128, HID], dt.float32, kind="ExternalInput")
    wq_d = nc.dram_tensor("wq", [128, HID], dt.float32, kind="ExternalInput")
    aw2_d = nc.dram_tensor("aw2", [128, HID], dt.float32, kind="ExternalInput")
    l1w_d = nc.dram_tensor("l1w", [128, HID], dt.float32, kind="ExternalInput")
    l2w_d = nc.dram_tensor("l2w", [128, HID], dt.float32, kind="ExternalInput")
    l3w_d = nc.dram_tensor("l3w", [128, HID], dt.float32, kind="ExternalInput")
    qb_d = nc.dram_tensor("qb", [128, 1], dt.float32, kind="ExternalInput")
    l1b_d = nc.dram_tensor("l1b", [128, 1], dt.float32, kind="ExternalInput")
    l3b_d = nc.dram_tensor("l3b", [128, 1], dt.float32, kind="ExternalInput")

    h_d = nc.dram_tensor("fh", [HROWS, 576], dt.float32, kind="Internal")
    a_d = nc.dram_tensor("fa", [HROWS, 64], dt.float32, kind="Internal")
    zb_d = nc.dram_tensor("zb", [R0, 1], dt.float32, kind="Internal")
    xn_d = nc.dram_tensor("xn", [XN_ROWS, HID], dt.float32,
                          kind="ExternalOutput")
    z_d = nc.dram_tensor("z", [R0, 1], dt.float32, kind="ExternalOutput")

    with TileContext(nc) as tc:
        with (
            tc.tile_pool(name="const", bufs=1) as cpool,
            tc.tile_pool(name="ps", bufs=2, space="PSUM") as pspool,
        ):
            ident = cpool.tile([128, 128], dt.float32)
            make_identity(nc, ident[:])
            wxm_sb = cpool.tile([128, HID], dt.float32)
            nc.sync.dma_start(wxm_sb[:], wxm_d[:, :])
            br_sb = cpool.tile([128, HID], dt.float32)
            nc.sync.dma_start(br_sb[:], br_d[:, :])
            wq_sb = cpool.tile([128, HID], dt.float32)
            nc.sync.dma_start(wq_sb[:], wq_d[:, :])
            aw2_sb = cpool.tile([128, HID], dt.float32)
            nc.sync.dma_start(aw2_sb[:], aw2_d[:, :])
            l1w_sb = cpool.tile([128, HID], dt.float32)
            nc.sync.dma_start(l1w_sb[:], l1w_d[:, :])
            l2w_sb = cpool.tile([128, HID], dt.float32)
            nc.sync.dma_start(l2w_sb[:], l2w_d[:, :])
            l3w_sb = cpool.tile([128, HID], dt.float32)
            nc.sync.dma_start(l3w_sb[:], l3w_d[:, :])
            qb_sb = cpool.tile([128, 1], dt.float32)
            nc.sync.dma_start(qb_sb[:], qb_d[:, :])
            l1b_sb = cpool.tile([128, 1], dt.float32)
            nc.sync.dma_start(l1b_sb[:], l1b_d[:, :])
            l3b_sb = cpool.tile([128, 1], dt.float32)
            nc.sync.dma_start(l3b_sb[:], l3b_d[:, :])
            pidx_sb = cpool.tile([128, SP], dt.int16)
            nc.sync.dma_start(pidx_sb[:], pidx_d[:, :])
            # sentinel rows: feat_h[R0] = zeros except js col = -1e30;
            # feat_a[R0] = 0
            srow = cpool.tile([1, 576], dt.float32)
            nc.vector.memset(srow[:], 0.0)
            nc.vector.memset(srow[:, 512:513], -1e30)
            nc.sync.dma_start(h_d[R0:R0 + 1, :], srow[:])
            nc.sync.dma_start(a_d[R0:R0 + 1, :], srow[:, 0:64])

            # ---- phase A: conv ----
            SCB = (128 * D0C) // 16
            with tc.tile_pool(name="conv", bufs=2) as wp:
                def conv_body(i):
                    ci = wp.tile([128, SCB], dt.int16, tag="ci")
                    nc.sync.dma_start(ci[:], cidx_d[:, ds(i * SCB, SCB)])
                    g = wp.tile([128, D0C, F], dt.float32, tag="g")
                    nc.gpsimd.dma_gather(
                        out_ap=g[:], in_ap=x_d[:, :], idxs_ap=ci[:],
                        num_idxs=128 * D0C, num_idxs_reg=128 * D0C,
                        elem_size=F, single_packet=False)
                    _tree_sum(nc, g, D0C,
                              lambda lo, cnt: g[:, lo:lo + cnt, :])
                    iv = wp.tile([128, 1], dt.float32, tag="iv")
                    nc.sync.dma_start(iv[:], invdeg_d[ds(i * 128, 128), :])
                    xm = wp.tile([128, 128], dt.float32, tag="xm")
                    nc.sync.dma_start(xm[:, 0:F], x_d[ds(i * 128, 128), :])
                    nc.vector.tensor_scalar_mul(xm[:, F:2 * F], g[:, 0, :],
                                                iv[:])
                    tp = pspool.tile([128, 128], dt.float32, tag="tp")
                    nc.tensor.transpose(tp[:], xm[:], ident[:])
                    lhsT = wp.tile([128, 128], dt.float32, tag="lhsT")
                    nc.vector.tensor_copy(lhsT[:], tp[:])
                    hps = pspool.tile([128, HID], dt.float32, tag="hps")
                    nc.tensor.matmul(hps[:], lhsT[:], wxm_sb[:],
                                     start=True, stop=True)
                    hsb = wp.tile([128, 576], dt.float32, tag="hsb")
                    nc.vector.tensor_add(
                        hsb[:, 0:HID], hps[:],
                        br_sb[:])
                    nc.vector.tensor_scalar_max(hsb[:, 0:HID], hsb[:, 0:HID],
                                                0.0)
                    tmp = wp.tile([128, HID], dt.float32, tag="tmp")
                    nc.vector.tensor_mul(tmp[:], hsb[:, 0:HID],
                                         aw2_sb[:])
                    nc.vector.tensor_reduce(hsb[:, 512:513], tmp[:],
                                            axis=mybir.AxisListType.X,
                                            op=mybir.AluOpType.add)
                    nc.sync.dma_start(h_d[ds(i * 128, 128), 0:513],
                                      hsb[:, 0:513])
                tc.For_i_unrolled(0, NB0, 1, conv_body, max_unroll=2)

            # ---- phase B: pool ----
            SPB = (128 * D0P) // 16
            with tc.tile_pool(name="pool", bufs=2) as wp:
                def pool_body(i):
                    g = wp.tile([128, D0P, 576], dt.float32, tag="g")
                    nc.gpsimd.dma_gather(
                        out_ap=g[:], in_ap=h_d[:, :],
                        idxs_ap=pidx_sb[:, ds(i * SPB, SPB)],
                        num_idxs=128 * D0P, num_idxs_reg=128 * D0P,
                        elem_size=576, single_packet=False)
                    xq = wp.tile([128, D0P // 2, HID], dt.float32, tag="xq")
                    _tree_max(nc, xq, g, D0P,
                              lambda lo, cnt: g[:, lo:lo + cnt, 0:HID],
                              lambda lo, cnt: xq[:, lo:lo + cnt, :])
                    tmp = wp.tile([128, HID], dt.float32, tag="tmp")
                    nc.vector.tensor_mul(tmp[:], xq[:, 0, :],
                                         wq_sb[:])
                    qs = wp.tile([128, 1], dt.float32, tag="qs")
                    nc.vector.tensor_reduce(qs[:], tmp[:],
                                            axis=mybir.AxisListType.X,
                                            op=mybir.AluOpType.add)
                    nc.vector.tensor_add(qs[:], qs[:], qb_sb[:])
                    # score = leaky_relu(qs + js)
                    sc = wp.tile([128, D0P], dt.float32, tag="sc")
                    jsv = g[:, :, 512:513].squeeze(2)
                    nc.vector.tensor_scalar_add(sc[:], jsv, qs[:])
                    sc2 = wp.tile([128, D0P], dt.float32, tag="sc2")
                    nc.vector.tensor_scalar_mul(sc2[:], sc[:], 0.2)
                    nc.vector.tensor_max(sc[:], sc[:], sc2[:])
                    m = wp.tile([128, 1], dt.float32, tag="m")
                    nc.vector.tensor_reduce(m[:], sc[:],
                                            axis=mybir.AxisListType.X,
                                            op=mybir.AluOpType.max)
                    nc.vector.tensor_scalar(sc[:], sc[:], m[:], None,
                                            op0=mybir.AluOpType.subtract)
                    nc.scalar.activation(sc[:], sc[:],
                                         mybir.ActivationFunctionType.Exp)
                    ssum = wp.tile([128, 1], dt.float32, tag="ssum")
                    nc.vector.tensor_reduce(ssum[:], sc[:],
                                            axis=mybir.AxisListType.X,
                                            op=mybir.AluOpType.add)
                    rec = wp.tile([128, 1], dt.float32, tag="rec")
                    nc.vector.reciprocal(rec[:], ssum[:])
                    nc.vector.tensor_scalar_mul(sc[:], sc[:], rec[:])
                    # xn = sum_s att_s * h_s  (scale slots in place, tree add)
                    gh = g[:, :, 0:HID]
                    nc.vector.tensor_mul(
                        gh, gh, sc[:].unsqueeze(2).to_broadcast(
                            [128, D0P, HID]))
                    _tree_sum(nc, g, D0P,
                              lambda lo, cnt: g[:, lo:lo + cnt, 0:HID])
                    xn = g[:, 0, 0:HID]
                    nc.sync.dma_start(xn_d[ds(i * 128, 128), :], xn)
                    # fitness scalars
                    nc.vector.tensor_mul(tmp[:], xn,
                                         l1w_sb[:])
                    av = wp.tile([128, 1], dt.float32, tag="av")
                    nc.vector.tensor_reduce(av[:], tmp[:],
                                            axis=mybir.AxisListType.X,
                                            op=mybir.AluOpType.add)
                    nc.sync.dma_start(a_d[ds(i * 128, 128), 0:1], av[:])
                    nc.vector.tensor_mul(tmp[:], xn,
                                         l2w_sb[:])
                    bv = wp.tile([128, 1], dt.float32, tag="bv")
                    nc.vector.tensor_reduce(bv[:], tmp[:],
                                            axis=mybir.AxisListType.X,
                                            op=mybir.AluOpType.add)
                    nc.vector.tensor_mul(tmp[:], xn,
                                         l3w_sb[:])
                    cv = wp.tile([128, 1], dt.float32, tag="cv")
                    nc.vector.tensor_reduce(cv[:], tmp[:],
                                            axis=mybir.AxisListType.X,
                                            op=mybir.AluOpType.add)
                    ct = wp.tile([128, 1], dt.float32, tag="ct")
                    nc.sync.dma_start(ct[:], cnt_d[ds(i * 128, 128), :])
                    # zb = c + l3b - cnt*b + cnt*l1b
                    zb = wp.tile([128, 1], dt.float32, tag="zb")
                    nc.vector.tensor_mul(zb[:], ct[:], bv[:])
                    nc.vector.tensor_sub(zb[:], cv[:], zb[:])
                    nc.vector.tensor_add(zb[:], zb[:], l3b_sb[:])
                    lb1 = wp.tile([128, 1], dt.float32, tag="lb1")
                    nc.vector.tensor_mul(lb1[:], ct[:], l1b_sb[:])
                    nc.vector.tensor_add(zb[:], zb[:], lb1[:])
                    nc.sync.dma_start(zb_d[ds(i * 128, 128), :], zb[:])
                tc.For_i_unrolled(0, NB0, 1, pool_body, max_unroll=2)

            # ---- phase C: fitness gather ----
            with tc.tile_pool(name="fit", bufs=2) as wp:
                def fit_body(i):
                    ga = wp.tile([128, D0P, 64], dt.float32, tag="ga")
                    nc.gpsimd.dma_gather(
                        out_ap=ga[:], in_ap=a_d[:, :],
                        idxs_ap=pidx_sb[:, ds(i * SPB, SPB)],
                        num_idxs=128 * D0P, num_idxs_reg=128 * D0P,
                        elem_size=64, single_packet=False)
                    zs = wp.tile([128, 1], dt.float32, tag="zs")
                    nc.vector.tensor_reduce(zs[:], ga[:, :, 0:1].squeeze(2),
                                            axis=mybir.AxisListType.X,
                                            op=mybir.AluOpType.add)
                    zb = wp.tile([128, 1], dt.float32, tag="zb2")
                    nc.sync.dma_start(zb[:], zb_d[ds(i * 128, 128), :])
                    nc.vector.tensor_add(zs[:], zs[:], zb[:])
                    nc.sync.dma_start(z_d[ds(i * 128, 128), :], zs[:])
                tc.For_i_unrolled(0, NB0, 1, fit_body, max_unroll=4)
    nc.compile()
    return nc


def _build_L12():
    """Layers 1/2: conv + pool + fitness over fixed-degree kNN graph."""
    B = _get_bass()
    bacc, mybir, TileContext = B["bacc"], B["mybir"], B["TileContext"]
    ds, make_identity = B["ds"], B["make_identity"]
    dt = mybir.dt
    F = HID
    SC = NB1 * ((128 * D1C) // 16)
    SP = NB1 * ((128 * D1P) // 16)
    HROWS = R1 + 128                    # sentinel = R1

    nc = bacc.Bacc("TRN2", target_bir_lowering=False)
    x_d = nc.dram_tensor("x", [XN_ROWS, F], dt.float32, kind="ExternalInput")
    xT_d = nc.dram_tensor("xT", [F, XT_COLS], dt.float32, kind="ExternalInput")
    cidx_d = nc.dram_tensor("cidx", [128, SC], dt.int16, kind="ExternalInput")
    pidx_d = nc.dram_tensor("pidx", [128, SP], dt.int16, kind="ExternalInput")
    invdeg_d = nc.dram_tensor("invdeg", [128, 1], dt.float32,
                              kind="ExternalInput")
    cnt_d = nc.dram_tensor("cnt", [128, 1], dt.float32, kind="ExternalInput")
    wr_d = nc.dram_tensor("wr", [128, 4, HID], dt.float32,
                          kind="ExternalInput")
    wl_d = nc.dram_tensor("wl", [128, 4, HID], dt.float32,
                          kind="ExternalInput")
    br_d = nc.dram_tensor("br", [128, HID], dt.float32, kind="ExternalInput")
    wq_d = nc.dram_tensor("wq", [128, HID], dt.float32, kind="ExternalInput")
    aw2_d = nc.dram_tensor("aw2", [128, HID], dt.float32, kind="ExternalInput")
    l1w_d = nc.dram_tensor("l1w", [128, HID], dt.float32, kind="ExternalInput")
    l2w_d = nc.dram_tensor("l2w", [128, HID], dt.float32, kind="ExternalInput")
    l3w_d = nc.dram_tensor("l3w", [128, HID], dt.float32, kind="ExternalInput")
    qb_d = nc.dram_tensor("qb", [128, 1], dt.float32, kind="ExternalInput")
    l1b_d = nc.dram_tensor("l1b", [128, 1], dt.float32, kind="ExternalInput")
    l3b_d = nc.dram_tensor("l3b", [128, 1], dt.float32, kind="ExternalInput")

    h_d = nc.dram_tensor("fh", [HROWS, 576], dt.float32, kind="Internal")
    a_d = nc.dram_tensor("fa", [HROWS, 64], dt.float32, kind="Internal")
    zb_d = nc.dram_tensor("zb", [R1, 1], dt.float32, kind="Internal")
    xn_d = nc.dram_tensor("xn", [XN_ROWS, HID], dt.float32,
                          kind="ExternalOutput")
    z_d = nc.dram_tensor("z", [R1, 1], dt.float32, kind="ExternalOutput")

    with TileContext(nc) as tc:
        with (
            tc.tile_pool(name="const", bufs=1) as cpool,
            tc.tile_pool(name="ps", bufs=2, space="PSUM") as pspool,
        ):
            ident = cpool.tile([128, 128], dt.float32)
            make_identity(nc, ident[:])
            wr_sb = cpool.tile([128, 4, HID], dt.float32)
            nc.sync.dma_start(wr_sb[:], wr_d[:, :, :])
            wl_sb = cpool.tile([128, 4, HID], dt.float32)
            nc.sync.dma_start(wl_sb[:], wl_d[:, :, :])
            br_sb = cpool.tile([128, HID], dt.float32)
            nc.sync.dma_start(br_sb[:], br_d[:, :])
            wq_sb = cpool.tile([128, HID], dt.float32)
            nc.sync.dma_start(wq_sb[:], wq_d[:, :])
            aw2_sb = cpool.tile([128, HID], dt.float32)
            nc.sync.dma_start(aw2_sb[:], aw2_d[:, :])
            l1w_sb = cpool.tile([128, HID], dt.float32)
            nc.sync.dma_start(l1w_sb[:], l1w_d[:, :])
            l2w_sb = cpool.tile([128, HID], dt.float32)
            nc.sync.dma_start(l2w_sb[:], l2w_d[:, :])
            l3w_sb = cpool.tile([128, HID], dt.float32)
            nc.sync.dma_start(l3w_sb[:], l3w_d[:, :])
            qb_sb = cpool.tile([128, 1], dt.float32)
            nc.sync.dma_start(qb_sb[:], qb_d[:, :])
            l1b_sb = cpool.tile([128, 1], dt.float32)
            nc.sync.dma_start(l1b_sb[:], l1b_d[:, :])
            l3b_sb = cpool.tile([128, 1], dt.float32)
            nc.sync.dma_start(l3b_sb[:], l3b_d[:, :])
            iv_sb = cpool.tile([128, 1], dt.float32)
            nc.sync.dma_start(iv_sb[:], invdeg_d[:, :])
            ct_sb = cpool.tile([128, 1], dt.float32)
            nc.sync.dma_start(ct_sb[:], cnt_d[:, :])
            cidx_sb = cpool.tile([128, SC], dt.int16)
            nc.sync.dma_start(cidx_sb[:], cidx_d[:, :])
            pidx_sb = cpool.tile([128, SP], dt.int16)
            nc.sync.dma_start(pidx_sb[:], pidx_d[:, :])
            srow = cpool.tile([1, 576], dt.float32)
            nc.vector.memset(srow[:], 0.0)
            nc.vector.memset(srow[:, 512:513], -1e30)
            nc.sync.dma_start(h_d[R1:R1 + 1, :], srow[:])
            nc.sync.dma_start(a_d[R1:R1 + 1, :], srow[:, 0:64])

            SCB = (128 * D1C) // 16
            SPB = (128 * D1P) // 16
            with tc.tile_pool(name="conv", bufs=2) as wp:
                def conv_body(i):
                    g = wp.tile([128, D1C, F], dt.float32, tag="g")
                    nc.gpsimd.dma_gather(
                        out_ap=g[:], in_ap=x_d[:, :],
                        idxs_ap=cidx_sb[:, ds(i * SCB, SCB)],
                        num_idxs=128 * D1C, num_idxs_reg=128 * D1C,
                        elem_size=F, single_packet=False)
                    _tree_sum(nc, g, D1C,
                              lambda lo, cnt: g[:, lo:lo + cnt, :])
                    mean = wp.tile([128, F], dt.float32, tag="mean")
                    nc.vector.tensor_scalar_mul(mean[:], g[:, 0, :], iv_sb[:])
                    hps = pspool.tile([128, HID], dt.float32, tag="hps")
                    xt = wp.tile([128, 4, 128], dt.float32, tag="xt")
                    nc.sync.dma_start(
                        xt[:], xT_d[:, ds(i * 128, 128)].rearrange(
                            "(c r) m -> r c m", c=4))
                    mt = wp.tile([128, 4, 128], dt.float32, tag="mt")
                    for c in range(4):
                        tp = pspool.tile([128, 128], dt.float32, tag="tp")
                        nc.tensor.transpose(tp[:],
                                            mean[:, c * 128:(c + 1) * 128],
                                            ident[:])
                        nc.vector.tensor_copy(mt[:, c, :], tp[:])
                    for c in range(4):
                        nc.tensor.matmul(hps[:], xt[:, c, :], wl_sb[:, c, :],
                                         start=(c == 0), stop=False)
                    for c in range(4):
                        nc.tensor.matmul(hps[:], mt[:, c, :], wr_sb[:, c, :],
                                         start=False, stop=(c == 3))
                    hsb = wp.tile([128, 576], dt.float32, tag="hsb")
                    nc.vector.tensor_add(
                        hsb[:, 0:HID], hps[:],
                        br_sb[:])
                    nc.vector.tensor_scalar_max(hsb[:, 0:HID], hsb[:, 0:HID],
                                                0.0)
                    tmp = wp.tile([128, HID], dt.float32, tag="tmp")
                    nc.vector.tensor_mul(tmp[:], hsb[:, 0:HID],
                                         aw2_sb[:])
                    nc.vector.tensor_reduce(hsb[:, 512:513], tmp[:],
                                            axis=mybir.AxisListType.X,
                                            op=mybir.AluOpType.add)
                    nc.sync.dma_start(h_d[ds(i * 128, 128), 0:513],
                                      hsb[:, 0:513])
                tc.For_i_unrolled(0, NB1, 1, conv_body, max_unroll=2)

            with tc.tile_pool(name="pool", bufs=2) as wp:
                def pool_body(i):
                    g = wp.tile([128, D1P, 576], dt.float32, tag="g")
                    nc.gpsimd.dma_gather(
                        out_ap=g[:], in_ap=h_d[:, :],
                        idxs_ap=pidx_sb[:, ds(i * SPB, SPB)],
                        num_idxs=128 * D1P, num_idxs_reg=128 * D1P,
                        elem_size=576, single_packet=False)
                    xq = wp.tile([128, D1P // 2, HID], dt.float32, tag="xq")
                    _tree_max(nc, xq, g, D1P,
                              lambda lo, cnt: g[:, lo:lo + cnt, 0:HID],
                              lambda lo, cnt: xq[:, lo:lo + cnt, :])
                    tmp = wp.tile([128, HID], dt.float32, tag="tmp")
                    nc.vector.tensor_mul(tmp[:], xq[:, 0, :],
                                         wq_sb[:])
                    qs = wp.tile([128, 1], dt.float32, tag="qs")
                    nc.vector.tensor_reduce(qs[:], tmp[:],
                                            axis=mybir.AxisListType.X,
                                            op=mybir.AluOpType.add)
                    nc.vector.tensor_add(qs[:], qs[:], qb_sb[:])
                    sc = wp.tile([128, D1P], dt.float32, tag="sc")
                    jsv = g[:, :, 512:513].squeeze(2)
                    nc.vector.tensor_scalar_add(sc[:], jsv, qs[:])
                    sc2 = wp.tile([128, D1P], dt.float32, tag="sc2")
                    nc.vector.tensor_scalar_mul(sc2[:], sc[:], 0.2)
                    nc.vector.tensor_max(sc[:], sc[:], sc2[:])
                    m = wp.tile([128, 1], dt.float32, tag="m")
                    nc.vector.tensor_reduce(m[:], sc[:],
                                            axis=mybir.AxisListType.X,
                                            op=mybir.AluOpType.max)
                    nc.vector.tensor_scalar(sc[:], sc[:], m[:], None,
                                            op0=mybir.AluOpType.subtract)
                    nc.scalar.activation(sc[:], sc[:],
                                         mybir.ActivationFunctionType.Exp)
                    ssum = wp.tile([128, 1], dt.float32, tag="ssum")
                    nc.vector.tensor_reduce(ssum[:], sc[:],
                                            axis=mybir.AxisListType.X,
                                            op=mybir.AluOpType.add)
                    rec = wp.tile([128, 1], dt.float32, tag="rec")
                    nc.vector.reciprocal(rec[:], ssum[:])
                    nc.vector.tensor_scalar_mul(sc[:], sc[:], rec[:])
                    gh = g[:, :, 0:HID]
                    nc.vector.tensor_mul(
                        gh, gh, sc[:].unsqueeze(2).to_broadcast(
                            [128, D1P, HID]))
                    _tree_sum(nc, g, D1P,
                              lambda lo, cnt: g[:, lo:lo + cnt, 0:HID])
                    xn = g[:, 0, 0:HID]
                    nc.sync.dma_start(xn_d[ds(i * 128, 128), :], xn)
                    nc.vector.tensor_mul(tmp[:], xn,
                                         l1w_sb[:])
                    av = wp.tile([128, 1], dt.float32, tag="av")
                    nc.vector.tensor_reduce(av[:], tmp[:],
                                            axis=mybir.AxisListType.X,
                                            op=mybir.AluOpType.add)
                    nc.sync.dma_start(a_d[ds(i * 128, 128), 0:1], av[:])
                    nc.vector.tensor_mul(tmp[:], xn,
                                         l2w_sb[:])
                    bv = wp.tile([128, 1], dt.float32, tag="bv")
                    nc.vector.tensor_reduce(bv[:], tmp[:],
                                            axis=mybir.AxisListType.X,
                                            op=mybir.AluOpType.add)
                    nc.vector.tensor_mul(tmp[:], xn,
                                         l3w_sb[:])
                    cv = wp.tile([128, 1], dt.float32, tag="cv")
                    nc.vector.tensor_reduce(cv[:], tmp[:],
                                            axis=mybir.AxisListType.X,
                                            op=mybir.AluOpType.add)
                    zb = wp.tile([128, 1], dt.float32, tag="zb")
                    nc.vector.tensor_mul(zb[:], ct_sb[:], bv[:])
                    nc.vector.tensor_sub(zb[:], cv[:], zb[:])
                    nc.vector.tensor_add(zb[:], zb[:], l3b_sb[:])
                    lb1 = wp.tile([128, 1], dt.float32, tag="lb1")
                    nc.vector.tensor_mul(lb1[:], ct_sb[:], l1b_sb[:])
                    nc.vector.tensor_add(zb[:], zb[:], lb1[:])
                    nc.sync.dma_start(zb_d[ds(i * 128, 128), :], zb[:])
                tc.For_i_unrolled(0, NB1, 1, pool_body, max_unroll=2)

            with tc.tile_pool(name="fit", bufs=2) as wp:
                def fit_body(i):
                    ga = wp.tile([128, D1P, 64], dt.float32, tag="ga")
                    nc.gpsimd.dma_gather(
                        out_ap=ga[:], in_ap=a_d[:, :],
                        idxs_ap=pidx_sb[:, ds(i * SPB, SPB)],
                        num_idxs=128 * D1P, num_idxs_reg=128 * D1P,
                        elem_size=64, single_packet=False)
                    zs = wp.tile([128, 1], dt.float32, tag="zs")
                    nc.vector.tensor_reduce(zs[:], ga[:, :, 0:1].squeeze(2),
                                            axis=mybir.AxisListType.X,
                                            op=mybir.AluOpType.add)
                    zb = wp.tile([128, 1], dt.float32, tag="zb2")
                    nc.sync.dma_start(zb[:], zb_d[ds(i * 128, 128), :])
                    nc.vector.tensor_add(zs[:], zs[:], zb[:])
                    nc.sync.dma_start(z_d[ds(i * 128, 128), :], zs[:])
                tc.For_i_unrolled(0, NB1, 1, fit_body, max_unroll=4)
    nc.compile()
    return nc


def _build_K():
    """Select (gather xn[perm]*fv -> x, xT, running max) + kNN scan."""
    B = _get_bass()
    bacc, mybir, TileContext = B["bacc"], B["mybir"], B["TileContext"]
    ds, make_identity = B["ds"], B["make_identity"]
    dt = mybir.dt
    SS = NB1 * ((128 * 1) // 16)   # select idx cols (1 slot per row)

    nc = bacc.Bacc("TRN2", target_bir_lowering=False)
    xn_d = nc.dram_tensor("xn", [XN_ROWS, HID], dt.float32,
                          kind="ExternalInput")
    sidx_d = nc.dram_tensor("sidx", [128, SS], dt.int16, kind="ExternalInput")
    fv_d = nc.dram_tensor("fv", [R1, 1], dt.float32, kind="ExternalInput")
    msk_d = nc.dram_tensor("msk", [R1, 1], dt.float32, kind="ExternalInput")
    qT_d = nc.dram_tensor("qT", [4, XT_COLS], dt.float32, kind="ExternalInput")
    cand_d = nc.dram_tensor("cand", [4, XT_COLS], dt.float32,
                            kind="ExternalInput")
    x_d = nc.dram_tensor("xo", [XN_ROWS, HID], dt.float32,
                         kind="ExternalOutput")
    xT_d = nc.dram_tensor("xT", [HID, XT_COLS], dt.float32,
                          kind="ExternalOutput")
    xsp_d = nc.dram_tensor("xsp", [128, HID], dt.float32,
                           kind="ExternalOutput")
    knn_d = nc.dram_tensor("knn", [R1, 16], dt.uint16, kind="ExternalOutput")

    with TileContext(nc) as tc:
        with (
            tc.tile_pool(name="const", bufs=1) as cpool,
            tc.tile_pool(name="ps", bufs=2, space="PSUM") as pspool,
        ):
            ident = cpool.tile([128, 128], dt.float32)
            make_identity(nc, ident[:])
            sidx_sb = cpool.tile([128, SS], dt.int16)
            nc.sync.dma_start(sidx_sb[:], sidx_d[:, :])
            cand_sb = cpool.tile([4, XT_COLS], dt.float32)
            nc.sync.dma_start(cand_sb[:], cand_d[:, :])
            runmax = cpool.tile([128, HID], dt.float32)
            nc.vector.memset(runmax[:], -1e30)

            SSB = 8   # (128*1)//16
            with tc.tile_pool(name="sel", bufs=2) as wp:
                def sel_body(i):
                    g = wp.tile([128, 1, HID], dt.float32, tag="g")
                    nc.gpsimd.dma_gather(
                        out_ap=g[:], in_ap=xn_d[:, :],
                        idxs_ap=sidx_sb[:, ds(i * SSB, SSB)],
                        num_idxs=128, num_idxs_reg=128,
                        elem_size=HID, single_packet=False)
                    fv = wp.tile([128, 1], dt.float32, tag="fv")
                    nc.sync.dma_start(fv[:], fv_d[ds(i * 128, 128), :])
                    xs = wp.tile([128, HID], dt.float32, tag="xs")
                    nc.vector.tensor_scalar_mul(xs[:], g[:, 0, :], fv[:])
                    nc.sync.dma_start(x_d[ds(i * 128, 128), :], xs[:])
                    mk = wp.tile([128, 1], dt.float32, tag="mk")
                    nc.sync.dma_start(mk[:], msk_d[ds(i * 128, 128), :])
                    xm = wp.tile([128, HID], dt.float32, tag="xm2")
                    nc.vector.tensor_scalar_add(xm[:], xs[:], mk[:])
                    nc.vector.tensor_max(runmax[:], runmax[:], xm[:])
                    for c in range(4):
                        tp = pspool.tile([128, 128], dt.float32, tag="tp")
                        nc.tensor.transpose(tp[:],
                                            xs[:, c * 128:(c + 1) * 128],
                                            ident[:])
                        tt = wp.tile([128, 128], dt.float32, tag="tt")
                        nc.vector.tensor_copy(tt[:], tp[:])
                        nc.sync.dma_start(
                            xT_d[c * 128:(c + 1) * 128, ds(i * 128, 128)],
                            tt[:])
                tc.For_i_unrolled(0, NB1, 1, sel_body, max_unroll=2)
            nc.sync.dma_start(xsp_d[:, :], runmax[:])

            with tc.tile_pool(name="knn", bufs=2) as wp:
                def knn_body(i):
                    qsb = wp.tile([4, 128], dt.float32, tag="q")
                    nc.sync.dma_start(qsb[:], qT_d[:, ds(i * 128, 128)])
                    row = wp.tile([128, XT_COLS], dt.float32, tag="row")
                    for ch in range(NCH):
                        dps = pspool.tile([128, 512], dt.float32, tag="d")
                        nc.tensor.matmul(dps[:], qsb[:],
                                         cand_sb[:, ch * 512:(ch + 1) * 512],
                                         start=True, stop=True)
                        nc.scalar.activation(
                            row[:, ch * 512:(ch + 1) * 512], dps[:],
                            mybir.ActivationFunctionType.Copy)
                    v8 = wp.tile([128, 8], dt.float32, tag="v8")
                    nc.vector.max(out=v8[:], in_=row[:])
                    i16 = wp.tile([128, 16], dt.uint16, tag="i16")
                    i8 = wp.tile([128, 8], dt.uint32, tag="i8")
                    nc.vector.max_index(i8[:], v8[:], row[:])
                    nc.vector.tensor_copy(i16[:, 0:8], i8[:])
                    nc.vector.match_replace(out=row[:], in_to_replace=v8[:],
                                            in_values=row[:], imm_value=-3e30)
                    v8b = wp.tile([128, 8], dt.float32, tag="v8b")
                    nc.vector.max(out=v8b[:], in_=row[:])
                    i8b = wp.tile([128, 8], dt.uint32, tag="i8b")
                    nc.vector.max_index(i8b[:], v8b[:], row[:])
                    nc.vector.tensor_copy(i16[:, 8:16], i8b[:])
                    nc.sync.dma_start(knn_d[ds(i * 128, 128), :], i16[:])
                tc.For_i_unrolled(0, NB1, 1, knn_body, max_unroll=2)
    nc.compile()
    return nc


# ----------------------------------------------------------------------------
# build/compile management (import-time warm-up)
# ----------------------------------------------------------------------------

_RUNNERS = {}
_BUILD_LOCK = threading.Lock()
_BUILD_THREADS = []


def _get_runner(name, builder):
    with _BUILD_LOCK:
        if name in _RUNNERS:
            return _RUNNERS[name]
    r = _Launcher(builder()).warm()
    with _BUILD_LOCK:
        _RUNNERS.setdefault(name, r)
    return _RUNNERS[name]


def _warm():
    try:
        B = _get_bass()
        jnp = B["jnp"]
        ncs = {}
        ncs["L0_%d" % D0C_DEFAULT] = _build_L0(D0C_DEFAULT)
        ncs["L12"] = _build_L12()
        ncs["K"] = _build_K()
        launchers = {name: _Launcher(nc) for name, nc in ncs.items()}
        # compile the programs and the donated-zeros broadcast kernels in
        # parallel (each is a neuronx-cc subprocess)
        shapes = {}
        for l in launchers.values():
            for av in l.out_avals:
                shapes[(av.shape, str(av.dtype))] = av
        ths = [threading.Thread(target=l.warm) for l in launchers.values()]
        ths += [threading.Thread(
            target=lambda a=av: jnp.zeros(a.shape, a.dtype).block_until_ready())
            for av in shapes.values()]
        for t in ths:
            t.start()
        for t in ths:
            t.join()
        with _BUILD_LOCK:
            for name, l in launchers.items():
                _RUNNERS.setdefault(name, l)
    except Exception:  # pragma: no cover - fallback path handles
        import traceback
        traceback.print_exc()


_BUILD_THREADS.append(threading.Thread(target=_warm, daemon=True))
_BUILD_THREADS[-1].start()


# ----------------------------------------------------------------------------
# numpy fallbacks (used only if the device path fails)
# ----------------------------------------------------------------------------

def _np_reference(x, pos, src, dst, W):
    f = _f32
    n = N0
    xs = []
    for i in range(L):
        wr, br, wl = W["wr"][i], W["br"][i], W["wl"][i]
        agg = np.zeros((n, x.shape[1]), f)
        np.add.at(agg, dst, x[src])
        deg = np.bincount(dst, minlength=n).astype(f)
        mean = agg / np.maximum(deg, 1)[:, None]
        h = np.maximum(mean @ wr + br + x @ wl, 0).astype(f)
        sl = np.arange(n)
        s_ = np.concatenate([src, sl])
        d_ = np.concatenate([dst, sl])
        xj = h[s_]
        xq = np.full((n, HID), -np.inf, f)
        np.maximum.at(xq, d_, xj)
        xq = (xq @ W["lw"][i] + W["lb"][i]).astype(f)
        aw, ab = W["aw"][i], W["ab"][i]
        score = (xq[d_] @ aw[:HID] + xj @ aw[HID:] + ab).astype(f)
        score = np.where(score > 0, score, f(0.2) * score).astype(f)
        smax = np.full(n, -np.inf, f)
        np.maximum.at(smax, d_, score)
        ex = np.exp(score - smax[d_])
        ssum = np.zeros(n, f)
        np.add.at(ssum, d_, ex)
        att = (ex / ssum[d_]).astype(f)
        xn = np.zeros((n, HID), f)
        np.add.at(xn, d_, xj * att[:, None])
        a = xn @ W["l1w"][i] + W["l1b"][i]
        b = xn @ W["l2w"][i]
        agg2 = np.zeros(n, f)
        np.add.at(agg2, d_, (a[s_] - b[d_]).astype(f))
        z = (agg2 + xn @ W["l3w"][i] + W["l3b"][i]).astype(f)
        k_keep = int(math.ceil(RATIO * n))
        fit64 = 1.0 / (1.0 + np.exp(-z.astype(np.float64)))
        perm = np.argpartition(-fit64, k_keep - 1)[:k_keep]
        fv = fit64[perm].astype(f)
        x = (xn[perm] * fv[:, None]).astype(f)
        xs.append(x.max(0))
        pos = pos[perm]
        n = k_keep
        if i < L - 1:
            k = 6 + 2 * i
            sq = np.sum(pos * pos, -1)
            dist = sq[:, None] + sq[None, :] - 2 * (pos @ pos.T)
            np.fill_diagonal(dist, np.inf)
            idx = np.argpartition(dist, k, 1)[:, :k]
            srt = np.take_along_axis(dist, idx, 1).argsort(1, kind="stable")
            idx = np.take_along_axis(idx, srt, 1)
            dst = np.repeat(np.arange(n), k)
            src = idx.reshape(-1)
    return xs


# ----------------------------------------------------------------------------
# kNN host validation
# ----------------------------------------------------------------------------

def _knn_from_cand(cand16, pos, k):
    """cand16: [n, 16] device max-index results (cols sorted by -dist).
    Returns tbl [n, k] of neighbor ids; falls back per-row when needed."""
    n = pos.shape[0]
    selfid = np.arange(n, dtype=np.int64)
    c = cand16.astype(np.int64)
    not_self = c != selfid[:, None]
    # positions of first k non-self entries per row
    cum = np.cumsum(not_self, 1)
    takec = (cum <= k) & not_self
    enough = cum[:, -1] >= k
    tbl = np.zeros((n, k), np.int64)
    rows_ok = np.flatnonzero(enough)
    # fill via argsort trick: order of selected cols preserved
    sel = np.where(takec, np.arange(16)[None, :], 99)
    ordcols = np.argsort(sel, 1, kind="stable")[:, :k]
    tbl = np.take_along_axis(c, ordcols, 1)
    # validity: unique and in range
    srt = np.sort(tbl, 1)
    dup = (srt[:, 1:] == srt[:, :-1]).any(1)
    oob = (tbl < 0).any(1) | (tbl >= n).any(1)
    bad = dup | oob | ~enough
    bad_rows = np.flatnonzero(bad)
    if len(bad_rows):
        sq = np.sum(pos * pos, 1)
        for i in bad_rows:
            d = sq + sq[i] - 2.0 * (pos @ pos[i])
            d[i] = np.inf
            idx = np.argpartition(d, k)[:k]
            tbl[i] = idx[np.argsort(d[idx], kind="stable")]
    return tbl


# ----------------------------------------------------------------------------
# main kernel
# ----------------------------------------------------------------------------

_EXEC_NS = []


def kernel(x, pos, edge_index, conv0_wr, conv0_br, conv0_wl, conv_wr, conv_br,
           conv_wl, pool_lin_w, pool_lin_b, pool_att_w, pool_att_b, le1_w,
           le1_b, le2_w, le3_w, le3_b, lin1_w, lin1_b, lin2_w, lin2_b):
    t_start = time.perf_counter()
    _EXEC_NS.clear()
    x = np.asarray(x, _f32)
    pos = np.asarray(pos, _f32)
    ei = np.asarray(edge_index).astype(np.int64)

    W = {
        "wr": [np.asarray(conv0_wr, _f32)] + [np.asarray(conv_wr[i], _f32)
                                              for i in range(L - 1)],
        "br": [np.asarray(conv0_br, _f32)] + [np.asarray(conv_br[i], _f32)
                                              for i in range(L - 1)],
        "wl": [np.asarray(conv0_wl, _f32)] + [np.asarray(conv_wl[i], _f32)
                                              for i in range(L - 1)],
        "lw": [np.asarray(pool_lin_w[i], _f32) for i in range(L)],
        "lb": [np.asarray(pool_lin_b[i], _f32) for i in range(L)],
        "aw": [np.asarray(pool_att_w[i], _f32) for i in range(L)],
        "ab": [float(pool_att_b[i]) for i in range(L)],
        "l1w": [np.asarray(le1_w[i], _f32) for i in range(L)],
        "l1b": [float(le1_b[i]) for i in range(L)],
        "l2w": [np.asarray(le2_w[i], _f32) for i in range(L)],
        "l3w": [np.asarray(le3_w[i], _f32) for i in range(L)],
        "l3b": [float(le3_b[i]) for i in range(L)],
    }
    try:
        xs = _device_forward(x, pos, ei, W)
    except Exception:
        import traceback
        traceback.print_exc()
        print("kernel: device path failed; numpy fallback")
        xs = _np_reference(x, pos, ei[0], ei[1], W)

    hcat = np.concatenate(xs)[None, :].astype(_f32)
    h1 = np.maximum(hcat @ np.asarray(lin1_w, _f32) +
                    np.asarray(lin1_b, _f32), 0)
    out = (h1 @ np.asarray(lin2_w, _f32) + np.asarray(lin2_b, _f32))
    dt_ns = int((time.perf_counter() - t_start) * 1e9)
    _EXEC_NS.append(("kernel", dt_ns))
    return out.astype(_f32)


def _layer_weights(W, i):
    """Pack per-layer pool/fitness weight vectors for the L programs."""
    lw, lb = W["lw"][i], W["lb"][i]
    aw, ab = W["aw"][i], W["ab"][i]
    wq = (lw @ aw[:HID]).astype(_f32)
    qb = float(lb @ aw[:HID] + ab)
    rep = lambda v: np.ascontiguousarray(
        np.broadcast_to(np.asarray(v, _f32), (128, HID)))
    return {
        "br": rep(W["br"][i]),
        "wq": rep(wq),
        "aw2": rep(aw[HID:]),
        "l1w": rep(W["l1w"][i]),
        "l2w": rep(W["l2w"][i]),
        "l3w": rep(W["l3w"][i]),
        "qb": _rep128(qb),
        "l1b": _rep128(W["l1b"][i]),
        "l3b": _rep128(W["l3b"][i]),
    }


def _device_forward(x, pos, ei, W):
    B = _get_bass()
    jax, jnp = B["jax"], B["jnp"]
    dev = jax.devices()[0]
    put = lambda a: jax.device_put(a, dev)
    src, dst = ei[0], ei[1]

    # ---------------- layer 0 host prep ----------------
    deg0 = np.bincount(dst, minlength=R0).astype(np.int64)
    D0C = max(int(deg0.max()), 1)
    name0 = "L0_%d" % D0C
    # kick off / reuse builds
    for th in _BUILD_THREADS:
        th.join()
    L0run = _RUNNERS.get(name0) or _get_runner(name0, lambda: _build_L0(D0C))
    L12run = _RUNNERS.get("L12") or _get_runner("L12", _build_L12)
    Krun = _RUNNERS.get("K") or _get_runner("K", _build_K)

    x0 = np.zeros((X0_ROWS, IN_CH), _f32)
    x0[:N0] = x
    x0_dev = put(x0)

    SENT0 = R0
    tblC, _ = _slot_table(src, dst, R0, D0C, SENT0)
    cidx0 = _idx_to_i16_tile(_slotmajor_list(tblC))
    tblP = np.concatenate(
        [np.arange(R0, dtype=np.int64)[:, None], tblC], 1)
    tblP[N0:, 0] = SENT0   # pad rows: no self slot
    pidx0 = _idx_to_i16_tile(_slotmajor_list(tblP))
    invdeg0 = (1.0 / np.maximum(deg0, 1.0)).astype(_f32)[:, None]
    cnt0 = (deg0 + 1).astype(_f32)[:, None]
    lw0 = _layer_weights(W, 0)
    wxm = np.zeros((128, HID), _f32)
    wxm[0:IN_CH] = W["wl"][0]
    wxm[IN_CH:2 * IN_CH] = W["wr"][0]

    in0 = {"x": x0_dev, "cidx": put(cidx0), "pidx": put(pidx0),
           "invdeg": put(invdeg0), "cnt": put(cnt0), "wxm": put(wxm)}
    in0.update({k: put(v) for k, v in lw0.items()})
    t0 = time.perf_counter()
    r0 = L0run(in0)
    z0 = np.asarray(r0["z"])[:N0, 0]
    _EXEC_NS.append(("L0", int((time.perf_counter() - t0) * 1e9)))

    xs_out = []
    feat_xn = r0["xn"]
    cur_pos = pos
    n_cur = N0
    for i in range(L):
        k_keep = int(math.ceil(RATIO * n_cur))
        z = z0
        # ---- host top-k ----
        perm = np.argpartition(-z, k_keep - 1)[:k_keep]
        fit = (1.0 / (1.0 + np.exp(-z[perm].astype(np.float64)))).astype(_f32)
        sel = np.zeros(R1, np.int64)
        sel[:k_keep] = perm
        fv = np.zeros((R1, 1), _f32)
        fv[:k_keep, 0] = fit
        msk = np.full((R1, 1), -1e30, _f32)
        msk[:k_keep] = 0.0
        cur_pos = cur_pos[perm]
        n_cur = k_keep
        # ---- kNN inputs ----
        if i < L - 1:
            kk = 6 + 2 * i
            sq = np.sum(cur_pos * cur_pos, 1, dtype=_f32)
            qT = np.zeros((4, XT_COLS), _f32)
            qT[0, :n_cur] = 2.0 * cur_pos[:, 0]
            qT[1, :n_cur] = 2.0 * cur_pos[:, 1]
            qT[2, :n_cur] = -1.0
            qT[3, :n_cur] = -sq
            cand = np.zeros((4, XT_COLS), _f32)
            cand[0, :n_cur] = cur_pos[:, 0]
            cand[1, :n_cur] = cur_pos[:, 1]
            cand[2, :n_cur] = sq
            cand[2, n_cur:] = 1e30
            cand[3, :] = 1.0
        else:
            kk = 0
            qT = np.zeros((4, XT_COLS), _f32)
            cand = np.zeros((4, XT_COLS), _f32)
        t0 = time.perf_counter()
        rK = Krun({"xn": feat_xn, "sidx": put(_idx_to_i16_tile(sel)),
                   "fv": put(fv), "msk": put(msk),
                   "qT": put(qT), "cand": put(cand)})
        xsp = np.asarray(rK["xsp"])
        xs_out.append(xsp.max(0))
        _EXEC_NS.append(("K%d" % i, int((time.perf_counter() - t0) * 1e9)))
        if i == L - 1:
            break
        cand16 = np.asarray(rK["knn"])[:n_cur]
        tbl = _knn_from_cand(cand16, cur_pos, kk)

        # ---- next layer tables ----
        SENT1 = R1
        tblC1 = np.full((R1, D1C), SENT1, np.int64)
        tblC1[:n_cur, :kk] = tbl
        cidx1 = _idx_to_i16_tile(_slotmajor_list(tblC1))
        tblP1 = np.concatenate(
            [np.arange(R1, dtype=np.int64)[:, None], tblC1], 1)
        tblP1[n_cur:, 0] = SENT1
        pidx1 = _idx_to_i16_tile(_slotmajor_list(tblP1))
        lwi = _layer_weights(W, i + 1)
        wr_c = np.ascontiguousarray(
            W["wr"][i + 1].reshape(4, 128, HID).transpose(1, 0, 2))
        wl_c = np.ascontiguousarray(
            W["wl"][i + 1].reshape(4, 128, HID).transpose(1, 0, 2))
        inL = {"x": rK["xo"], "xT": rK["xT"],
               "cidx": put(cidx1), "pidx": put(pidx1),
               "invdeg": _rep128(1.0 / kk), "cnt": _rep128(kk + 1),
               "wr": put(wr_c), "wl": put(wl_c)}
        inL["invdeg"] = put(inL["invdeg"])
        inL["cnt"] = put(inL["cnt"])
        inL.update({k: put(v) for k, v in lwi.items()})
        t0 = time.perf_counter()
        rL = L12run(inL)
        z0 = np.asarray(rL["z"])[:n_cur, 0]
        _EXEC_NS.append(("L%d" % (i + 1),
                         int((time.perf_counter() - t0) * 1e9)))
        feat_xn = rL["xn"]
    return xs_out


def total_exec_ns():
    return sum(v for k, v in _EXEC_NS if k == "kernel")


def exec_breakdown():
    return list(_EXEC_NS)
